# revision 43
# baseline (speedup 1.0000x reference)
# Trainium2 Bass kernel for nn_AttentionLayer_69380901699611.
#
# Full-input contract: kernel(**inputs) takes the unsharded numpy inputs and
# returns the full [B, F, HIDDEN] output. Internally the work is sharded over
# 8 NeuronCores as (batch x head-group): core c handles batch c//4 and heads
# [4*(c%4), 4*(c%4)+4). Each core computes a partial output projection over
# its 4 heads; the host sums the 4 partials per batch.
#
# Device kernel layout (per core):
#   qT, kT      [NH=256 part-chunks, F/T] bf16 (head-dim on partitions)
#   v           [T part, NH] bf16 with an appended ones column per head (the
#               softmax denominator falls out of the attn matmul for free)
#   scores^T    [T part, F free] fp32 psum = kT_chunk.T @ qT_chunk (K=64 pairs
#               on row groups 0-63 / 64-127)
#   softmax     exp on ACT (psum -> bf16), then multiply by exp(bias)^T
#               (precomputed on host, bf16) on DVE in 2x bf16 mode.
#               No max-subtraction needed: |logits| <~ 12.
#   attn        accumulated as [F-tile part, (h, H+1) free] fp32 psum: the
#               attn matmuls use pt (scores^T) as the STATIONARY operand and
#               v as the moving operand, so each matmul is N=65 wide with all
#               128 output partitions used -- half the PE rows of the
#               [nh part, F free] orientation.
#   normalize   per-partition: reciprocal of the denominator column then one
#               broadcast tensor_tensor per F-tile (DVE), bf16 out.
#   transpose   [f, nh] -> [nh, f] via the XBAR DMA transpose (16x128 tiles,
#               runs on the DMA engines, not PE/DVE/ACT).
#   out         attnT.T @ wo per F-tile; psum DMA'd straight to DRAM.

import numpy as np

B, F, T, C = 2, 2048, 2048, 1024
HEADS, DEPTH = 16, 64
N_CORES = 8
HG = 4  # head-groups; heads per group = HEADS // HG = 4
# fp8e4m3 DoubleRow score matmuls: halves score-matmul PE time but raises
# rel err from ~4e-3 to ~1.7e-2 (gate is 2e-2). Off = safe margin.
USE_FP8_SCORES = False


def build_attention_nc(C=1024, F=2048, T=2048, NHEADS=4, H=64, fc_w=512,
                       use_fp8=False, debug_taps=False):
    import concourse.tile as tile
    import concourse.mybir as mybir
    from concourse import bacc

    P = 128
    NH = NHEADS * H          # local heads * depth (256)
    KC = C // P              # contraction subtiles for projections (8)
    NFC = F // fc_w          # F chunks (4)
    NTT = T // P             # T tiles (16)
    NHC = NH // P            # NH chunks of 128 partitions (2)
    FPC = fc_w // P          # F tiles per F chunk (4)
    assert NHC * 2 == NHEADS and H == 64, "layout assumes 2 heads per NH chunk"
    f32 = mybir.dt.float32
    bf16 = mybir.dt.bfloat16
    fp8 = mybir.dt.float8e4
    DR = mybir.MatmulPerfMode.DoubleRow
    Exp = mybir.ActivationFunctionType.Exp
    Mult = mybir.AluOpType.mult

    nc = bacc.Bacc("TRN2", target_bir_lowering=False, debug=False, name="attn69")

    qT_d = nc.dram_tensor("qT", [C, F], bf16, kind="ExternalInput")
    sT_d = nc.dram_tensor("sT", [C, T], bf16, kind="ExternalInput")
    eb_d = nc.dram_tensor("ebT", [T, F], bf16, kind="ExternalInput")
    id_d = nc.dram_tensor("ident", [P, P], bf16, kind="ExternalInput")
    wq_d = nc.dram_tensor("wq", [C, NH], bf16, kind="ExternalInput")
    wk_d = nc.dram_tensor("wk", [C, NH], bf16, kind="ExternalInput")
    wv_d = nc.dram_tensor("wv", [C, NH], bf16, kind="ExternalInput")
    wo_d = nc.dram_tensor("wo", [NH, C], bf16, kind="ExternalInput")
    out_d = nc.dram_tensor("out_p", [F, C], f32, kind="ExternalOutput")

    with tile.TileContext(nc) as tc:
        with (
            tc.tile_pool(name="constp", bufs=1) as constp,
            tc.tile_pool(name="persist", bufs=1) as persist,
            tc.tile_pool(name="actp", bufs=4) as actp,
            tc.tile_pool(name="biasp", bufs=6) as biasp,
            tc.tile_pool(name="ptp", bufs=3) as ptp,
            tc.tile_pool(name="smallp", bufs=4) as smallp,
            tc.tile_pool(name="normp", bufs=6) as normp,
            tc.tile_pool(name="psA", bufs=4, space="PSUM") as psA,
            tc.tile_pool(name="psS", bufs=2, space="PSUM") as psS,
        ):
            # ---------------- weights (tiles only; DMAs ordered below) -------
            wq_sb = constp.tile([P, KC, NH], bf16, name="wq_sb")
            wk_sb = constp.tile([P, KC, NH], bf16, name="wk_sb")
            wv_sb = constp.tile([P, KC, NH], bf16, name="wv_sb")
            wo_sb = constp.tile([P, NHC, C], bf16, name="wo_sb")

            # ---------------- persistent activations ----------------
            # fp8 mode: q/k live in fp8e4m3 with the DoubleRow layout --
            # partition 32*h + p holds head h's contraction rows p and 32+p
            # (kt the second dim); wq/wk columns are pre-permuted on the host
            # so the projection psum lands in exactly this partition order.
            # bf16 mode: q/k live as [head-dim part, pair, F] like v.
            qk_dt = fp8 if use_fp8 else bf16
            qT8 = persist.tile([P, 2, F], qk_dt, name="qT8")
            kT8 = persist.tile([P, 2, T], qk_dt, name="kT8")
            v_sb = persist.tile([P, NTT, NHEADS, H + 1], bf16, name="v_sb")
            attnT_sb = persist.tile([P, NHC, F], bf16, name="attnT_sb")
            # ones column for the softmax denominator (cols 0..H-1 overwritten
            # by the v projection; only col H needs initializing)
            ones1 = nc.const_aps.aps[(f32, 1.0)]
            nc.scalar.copy(
                v_sb[:, :, :, H:H + 1],
                ones1[:, None, None, :].to_broadcast((P, NTT, NHEADS, 1)))

            # ---------------- q projection (emitted per F chunk) ----------------
            # depth**-0.5 is folded into wq on the host, so this is a plain
            # psum->sbuf copy (DVE, keeping ACT free for the exps).
            qT_r = qT_d.ap().rearrange("(ko p) f -> p ko f", p=P)
            sT_r = sT_d.ap().rearrange("(ko p) t -> p ko t", p=P)
            t_per_chunk = fc_w // P

            def load_q(fc):
                qa = actp.tile([P, KC, fc_w], bf16, tag="act", name="qa")
                nc.sync.dma_start(qa[:], qT_r[:, :, fc * fc_w:(fc + 1) * fc_w])
                return qa

            def load_s(sc):
                sa = actp.tile([P, KC, fc_w], bf16, tag="act", name="sa")
                nc.sync.dma_start(sa[:], sT_r[:, :, sc * fc_w:(sc + 1) * fc_w])
                return sa

            def q_proj(fc, qa=None):
                if qa is None:
                    qa = load_q(fc)
                for m in range(NHC):
                    psq = psA.tile([P, 512], f32, tag="bank", name="psq")
                    for k in range(KC):
                        nc.tensor.matmul(
                            psq[:, :fc_w],
                            lhsT=wq_sb[:, k, m * P:(m + 1) * P],
                            rhs=qa[:, k, :],
                            start=(k == 0), stop=(k == KC - 1))
                    nc.vector.tensor_copy(
                        qT8[:, m, fc * fc_w:(fc + 1) * fc_w], psq[:, :fc_w])

            # ---------------- k and v projections ----------------
            def k_proj(sc, sa):
                for m in range(NHC):
                    psk = psA.tile([P, 512], f32, tag="bank", name="psk")
                    for k in range(KC):
                        nc.tensor.matmul(
                            psk[:, :fc_w],
                            lhsT=wk_sb[:, k, m * P:(m + 1) * P],
                            rhs=sa[:, k, :],
                            start=(k == 0), stop=(k == KC - 1))
                    nc.vector.tensor_copy(kT8[:, m, sc * fc_w:(sc + 1) * fc_w], psk[:, :fc_w])

            def v_proj(sc, sa):
                for tl in range(t_per_chunk):
                    tt = sc * t_per_chunk + tl
                    psv = psA.tile([P, 512], f32, tag="bank", name="psv")
                    for k in range(KC):
                        nc.tensor.matmul(
                            psv[:, :NH],
                            lhsT=sa[:, k, tl * P:(tl + 1) * P],
                            rhs=wv_sb[:, k, :],
                            start=(k == 0), stop=(k == KC - 1))
                    nc.vector.tensor_copy(
                        v_sb[:, tt, :, 0:H],
                        psv[:, :NH].rearrange("p (h x) -> p h x", h=NHEADS))

            # ------------- attention main loop (software-pipelined) -------------
            # Chunk fc's softmax stream (ST matmuls -> exp -> *exp(bias))
            # produces NTT pt tiles; chunk fc-1's attention accumulation,
            # normalize, and output projection are interleaved with it. Chunk
            # 0's stream overlaps the k/v projection prefix, so ACT/DVE are
            # busy during the PE-dense projection phase and across chunk
            # boundaries.
            pt_store = {}
            bias_store = {}
            eb_r = eb_d.ap().rearrange("(tg p) f -> p tg f", p=P)

            def load_b4(fc, q):
                # one bias DMA per quarter chunk (four t-tiles)
                b4 = biasp.tile([P, 4, fc_w], bf16, tag="bias", name="b4")
                nc.sync.dma_start(
                    b4[:], eb_r[:, 4 * q:4 * q + 4, fc * fc_w:(fc + 1) * fc_w])
                bias_store[(fc, q)] = b4

            def produce(fc, tt):
                fsl = slice(fc * fc_w, (fc + 1) * fc_w)
                if (fc, tt // 4) not in bias_store:
                    load_b4(fc, tt // 4)
                bias_t = bias_store[(fc, tt // 4)][:, tt % 4, :]
                if tt % 4 == 3:
                    del bias_store[(fc, tt // 4)]
                pt4 = ptp.tile([P, NHEADS, fc_w], bf16, tag="pt", bufs=20, name="pt4")
                for pair in range(NHC):
                    st2 = psS.tile([P, 2, 512], f32, tag="st", name="st2")
                    for j in range(2):
                        h = 2 * pair + j
                        if use_fp8:
                            nc.tensor.matmul(
                                st2[:, j, :fc_w],
                                lhsT=kT8[32 * h:32 * h + 32, :, tt * P:(tt + 1) * P],
                                rhs=qT8[32 * h:32 * h + 32, :, fsl],
                                start=True, stop=True, perf_mode=DR,
                                tile_position=(32 * h, 0))
                        else:
                            off = j * H
                            nc.tensor.matmul(
                                st2[:, j, :fc_w],
                                lhsT=kT8[off:off + H, pair, tt * P:(tt + 1) * P],
                                rhs=qT8[off:off + H, pair, fsl],
                                start=True, stop=True)
                    # exp(S^T), psum fp32 -> bf16
                    nc.scalar.activation(
                        pt4[:, 2 * pair:2 * pair + 2, :], st2[:, :, :fc_w], Exp)
                # multiply by exp(bias)^T tile, bf16 2x mode; bias is
                # broadcast over the head dim (stride-0): one instr per tile
                nc.vector.tensor_mul(
                    pt4[:], pt4[:],
                    bias_t[:, None, :].to_broadcast((P, NHEADS, fc_w)))
                pt_store[(fc, tt)] = pt4

            def consume(fc, tt, at_tiles):
                # at_tiles[ft] accumulates [128 f, (h, H+1)] for F-tile ft;
                # pt is the stationary operand so each matmul is only N=65.
                # The bank is pre-zeroed by alloc_at and every matmul runs
                # with start=False: a start=True in a bank discards any other
                # open accumulation group's partials there (measured on HW),
                # so per-head start flags cannot share a bank.
                pt4 = pt_store.pop((fc, tt))
                for ft in range(FPC):
                    at = at_tiles[ft]
                    for h in range(NHEADS):
                        nc.tensor.matmul(
                            at[:, h, :],
                            lhsT=pt4[:, h, ft * P:(ft + 1) * P],
                            rhs=v_sb[:, tt, h, :],
                            start=False, stop=(tt == NTT - 1),
                            skip_group_check=True)

            cw = min(512, C)

            def norm_ft(fc, ft, at):
                # normalize: attn[f, h, :] = acc[f, h, 0:H] / acc[f, h, H]
                rec = smallp.tile([P, NHEADS], f32, tag="rec", name="rec")
                nc.vector.reciprocal(rec[:], at[:, :, H])
                an = normp.tile([P, NHEADS, H], bf16, tag="an", name="an")
                nc.vector.tensor_tensor(
                    an[:], at[:, :, 0:H],
                    rec[:, :, None].to_broadcast((P, NHEADS, H)), Mult)
                return an

            def tail_ft(fc, ft, an):
                # XBAR-DMA-transpose the [128 f, 128 nh] blocks into attnT_sb,
                # then this F-tile's output projection
                ftA = fc * FPC + ft
                for m in range(NHC):
                    nc.sync.dma_start_transpose(
                        attnT_sb[:, m, ftA * P:(ftA + 1) * P],
                        an[:, 2 * m:2 * m + 2, :])
                for cc in range(C // cw):
                    pso = psA.tile([P, 512], f32, tag="bank", name="pso")
                    for m in range(NHC):
                        nc.tensor.matmul(
                            pso[:, :cw],
                            lhsT=attnT_sb[:, m, ftA * P:(ftA + 1) * P],
                            rhs=wo_sb[:, m, cc * cw:(cc + 1) * cw],
                            start=(m == 0), stop=(m == NHC - 1))
                    ot = normp.tile([P, 512], f32, tag="o", name="ot")
                    nc.vector.tensor_copy(ot[:, :cw], pso[:, :cw])
                    nc.sync.dma_start(
                        out_d.ap()[ftA * P:(ftA + 1) * P, cc * cw:(cc + 1) * cw],
                        ot[:, :cw])

            def make_piece(fc, ft, cc, an):
                # one drip-feedable slice of finish: the F-tile's transposes
                # (first slice only) plus one outproj psum + copy + store
                def piece():
                    ftA = fc * FPC + ft
                    if cc == 0:
                        for m in range(NHC):
                            nc.sync.dma_start_transpose(
                                attnT_sb[:, m, ftA * P:(ftA + 1) * P],
                                an[:, 2 * m:2 * m + 2, :])
                    pso = psA.tile([P, 512], f32, tag="bank", name="pso")
                    for m in range(NHC):
                        nc.tensor.matmul(
                            pso[:, :cw],
                            lhsT=attnT_sb[:, m, ftA * P:(ftA + 1) * P],
                            rhs=wo_sb[:, m, cc * cw:(cc + 1) * cw],
                            start=(m == 0), stop=(m == NHC - 1))
                    ot = normp.tile([P, 512], f32, tag="o", bufs=4, name="ot")
                    nc.vector.tensor_copy(ot[:, :cw], pso[:, :cw])
                    nc.sync.dma_start(
                        out_d.ap()[ftA * P:(ftA + 1) * P, cc * cw:(cc + 1) * cw],
                        ot[:, :cw])
                return piece

            # PE p-state warm-up: ~45 dummy matmuls on scratch data keep the
            # PE continuously busy from t~0.7us so the first real projection
            # matmuls run at the full 2.4 GHz clock instead of 0.65/1.2 GHz.
            scr = smallp.tile([P, 512], bf16, tag="scr", name="scr")
            nc.gpsimd.memset(scr[:], 0.0)
            pswu = psA.tile([P, 512], f32, tag="bank", name="pswu")
            for _ in range(45):
                nc.tensor.matmul(
                    pswu[:, :128], lhsT=scr[:, 0:128], rhs=scr[:, 128:256],
                    start=True, stop=True, skip_group_check=True)

            # chunk 0 production rides along with the k/v projections.
            # DMA issue order is tuned for the serial DMA pipe: k-proj inputs
            # first (sa half, wk), then q (qa, wq), then the first bias
            # quarter and wv; wo (needed ~60us in) goes last.
            sa0 = actp.tile([P, KC, fc_w], bf16, tag="act", name="sa0")
            nc.sync.dma_start(sa0[:, 0:KC // 2, :], sT_r[:, 0:KC // 2, 0:fc_w])
            nc.sync.dma_start(
                wk_sb[:], wk_d.ap().rearrange("(ko p) m -> p ko m", p=P))
            nc.sync.dma_start(sa0[:, KC // 2:, :], sT_r[:, KC // 2:, 0:fc_w])
            qa0 = actp.tile([P, KC, fc_w], bf16, tag="act", name="qa0")
            nc.sync.dma_start(qa0[:, 0:KC // 2, :], qT_r[:, 0:KC // 2, 0:fc_w])
            nc.sync.dma_start(
                wq_sb[:], wq_d.ap().rearrange("(ko p) m -> p ko m", p=P))
            nc.sync.dma_start(qa0[:, KC // 2:, :], qT_r[:, KC // 2:, 0:fc_w])
            load_b4(0, 0)
            nc.sync.dma_start(
                wv_sb[:], wv_d.ap().rearrange("(ko p) m -> p ko m", p=P))
            nc.sync.dma_start(
                wo_sb[:], wo_d.ap().rearrange("(ko p) m -> p ko m", p=P))
            id_sb = constp.tile([P, P], bf16, name="id_sb")
            nc.sync.dma_start(id_sb[:], id_d.ap())

            # emission order puts each source chunk's k-projection and the
            # score/exp stream before its v-projection (v is only needed by
            # the much-later consume stage)
            sa_next = None
            for sc in range(T // fc_w):
                sa = sa0 if sc == 0 else sa_next
                k_proj(sc, sa)
                if sc == 0:
                    q_proj(0, qa0)
                if sc + 1 < T // fc_w:
                    load_b4(0, sc + 1)
                    sa_next = load_s(sc + 1)
                for tl in range(t_per_chunk):
                    produce(0, sc * t_per_chunk + tl)
                v_proj(sc, sa)
            for fc in range(1, NFC):
                q_proj(fc)

            def alloc_at():
                tiles = []
                for ft in range(FPC):
                    at = psA.tile([P, NHEADS, H + 1], f32, tag="bank", name=f"at{ft}")
                    nc.vector.memset(at[:], 0.0)
                    tiles.append(at)
                return tiles

            # Greedy catch-up pipeline: the produce stream runs continuously
            # (chunks 1..NFC-1) while consumes trail, draining at up to two
            # units per produce slot. A unit is either a consume tile or a
            # finish "piece" (transpose + one outproj psum): finish work is
            # drip-fed across slots so it never bursts the PE at a chunk
            # boundary, and piece psums recycle the accumulator banks BEFORE
            # the next chunk's memsets claim them (pool request order).
            at_cur = alloc_at()  # chunk 0 accumulators
            c = 0          # global consume pointer (tile index)
            produced = NTT  # chunk 0 fully produced in the prefix
            pending = []   # queued finish pieces
            need_alloc = False

            def emit_slot_work(budget):
                nonlocal c, at_cur, need_alloc
                while budget > 0:
                    if pending:
                        pending.pop(0)()
                        budget -= 1
                        continue
                    if need_alloc:
                        at_cur = alloc_at()
                        need_alloc = False
                    if at_cur is None or c >= produced - 1:
                        break
                    cfc, ctt = divmod(c, NTT)
                    consume(cfc, ctt, at_cur)
                    c += 1
                    budget -= 1
                    if ctt == NTT - 1:
                        ans = [norm_ft(cfc, ft, at_cur[ft]) for ft in range(FPC)]
                        for ft in range(FPC):
                            for cc in range(C // cw):
                                pending.append(make_piece(cfc, ft, cc, ans[ft]))
                        at_cur = None
                        need_alloc = cfc + 1 < NFC

            for fc in range(1, NFC):
                for tt in range(NTT):
                    produce(fc, tt)
                    produced += 1
                    emit_slot_work(2)
            # Tail: flush pending pieces and stragglers, then drain the last
            # chunk F-tile-major so each F-tile's normalize/transpose/outproj
            # overlaps the next F-tile's accumulation matmuls.
            while pending:
                pending.pop(0)()
            if need_alloc:
                at_cur = alloc_at()
                need_alloc = False
            while c < (NFC - 1) * NTT:
                cfc, ctt = divmod(c, NTT)
                consume(cfc, ctt, at_cur)
                c += 1
                if ctt == NTT - 1:
                    ans = [norm_ft(cfc, ft, at_cur[ft]) for ft in range(FPC)]
                    for ft in range(FPC):
                        tail_ft(cfc, ft, ans[ft])
                    at_cur = alloc_at()
            last = NFC - 1
            rem = [divmod(g, NTT)[1] for g in range(c, NFC * NTT)]
            pts = {tt: pt_store.pop((last, tt)) for tt in rem}
            for ft in range(FPC):
                at = at_cur[ft]
                for tt in rem:
                    for h in range(NHEADS):
                        nc.tensor.matmul(
                            at[:, h, :],
                            lhsT=pts[tt][:, h, ft * P:(ft + 1) * P],
                            rhs=v_sb[:, tt, h, :],
                            start=False, stop=(tt == rem[-1]),
                            skip_group_check=True)
                an = norm_ft(last, ft, at)
                # fast tail: PE transposes (identity matmul) + ACT copies
                # keep the last chunk's critical chain off the serial
                # HWDGE/DMA pipe; output DMA batched to one per F-tile
                ftA = last * FPC + ft
                pst = psA.tile([P, 512], f32, tag="bank", name="pst")
                pst_b = pst[:].bitcast(bf16)
                for m in range(NHC):
                    nc.tensor.matmul(
                        pst_b[:, m * P:(m + 1) * P],
                        lhsT=an[:, 2 * m:2 * m + 2, :],
                        rhs=id_sb[:],
                        start=True, stop=True, is_transpose=True,
                        skip_group_check=True)
                    nc.scalar.copy(
                        attnT_sb[:, m, ftA * P:(ftA + 1) * P],
                        pst_b[:, m * P:(m + 1) * P])
                ot = normp.tile([P, 2, 512], f32, tag="o2", bufs=2, name="ot2")
                for cc in range(C // cw):
                    pso = psA.tile([P, 512], f32, tag="bank", name="pso")
                    for m in range(NHC):
                        nc.tensor.matmul(
                            pso[:, :cw],
                            lhsT=attnT_sb[:, m, ftA * P:(ftA + 1) * P],
                            rhs=wo_sb[:, m, cc * cw:(cc + 1) * cw],
                            start=(m == 0), stop=(m == NHC - 1))
                    nc.vector.tensor_copy(ot[:, cc, :cw], pso[:, :cw])
                nc.sync.dma_start(
                    out_d.ap()[ftA * P:(ftA + 1) * P, :], ot[:])

    nc.compile()
    return nc


_CACHE = {}


def _get_nc():
    if "nc" not in _CACHE:
        _CACHE["nc"] = build_attention_nc(C=C, F=F, T=T, NHEADS=HEADS // HG,
                                          H=DEPTH, use_fp8=USE_FP8_SCORES)
    return _CACHE["nc"]


def kernel(query_input, source_input, bias, wq, wk, wv, wo, **run_kwargs):
    import ml_dtypes
    from concourse.bass_utils import run_bass_kernel_spmd

    bf = ml_dtypes.bfloat16
    q = np.asarray(query_input, dtype=np.float32)
    s = np.asarray(source_input, dtype=np.float32)
    b = np.asarray(bias, dtype=np.float32)
    scale = float(DEPTH) ** -0.5
    wq2 = np.asarray(wq, dtype=np.float32).reshape(C, HEADS * DEPTH) * scale
    wk2 = np.asarray(wk, dtype=np.float32).reshape(C, HEADS * DEPTH)
    wv2 = np.asarray(wv, dtype=np.float32).reshape(C, HEADS * DEPTH)
    wo2 = np.asarray(wo, dtype=np.float32).reshape(HEADS * DEPTH, C)

    qT = [np.ascontiguousarray(q[i].T).astype(bf) for i in range(B)]
    sT = [np.ascontiguousarray(s[i].T).astype(bf) for i in range(B)]
    ebT = np.exp(np.ascontiguousarray(b[0, 0].T)).astype(bf)

    nhl = (HEADS // HG) * DEPTH  # NH columns per core (256)
    # DoubleRow column permutation: psum partition pi of matmul group m must
    # hold nh = (pi//32)*64 + 32*m + pi%32 (head pi//32, k-tile m, row pi%32)
    pi = np.arange(128)
    if USE_FP8_SCORES:
        dr_perm = np.concatenate([(pi // 32) * 64 + 32 * m + pi % 32 for m in (0, 1)])
    else:
        dr_perm = np.arange(2 * 128)
    in_maps = []
    for c in range(N_CORES):
        bi, hg = c // HG, c % HG
        sl = slice(hg * nhl, (hg + 1) * nhl)
        in_maps.append({
            "qT": qT[bi],
            "sT": sT[bi],
            "ebT": ebT,
            "ident": np.eye(128, dtype=np.float32).astype(bf),
            "wq": np.ascontiguousarray(wq2[:, sl][:, dr_perm]).astype(bf),
            "wk": np.ascontiguousarray(wk2[:, sl][:, dr_perm]).astype(bf),
            "wv": np.ascontiguousarray(wv2[:, sl]).astype(bf),
            "wo": np.ascontiguousarray(wo2[sl, :]).astype(bf),
        })

    nc = _get_nc()
    res = run_bass_kernel_spmd(nc, in_maps, core_ids=list(range(N_CORES)), **run_kwargs)
    _CACHE["last_results"] = res

    out = np.empty((B, F, C), np.float32)
    for bi in range(B):
        acc = res.results[bi * HG]["out_p"].astype(np.float32)
        for hg in range(1, HG):
            acc = acc + res.results[bi * HG + hg]["out_p"]
        out[bi] = acc
    return out


# revision 44
# speedup vs baseline: 1.0256x; 1.0256x over previous
# Trainium2 Bass kernel for nn_AttentionLayer_69380901699611.
#
# Full-input contract: kernel(**inputs) takes the unsharded numpy inputs and
# returns the full [B, F, HIDDEN] output. Internally the work is sharded over
# 8 NeuronCores as (batch x head-group): core c handles batch c//4 and heads
# [4*(c%4), 4*(c%4)+4). Each core computes a partial output projection over
# its 4 heads; the host sums the 4 partials per batch.
#
# Device kernel layout (per core):
#   qT, kT      [NH=256 part-chunks, F/T] bf16 (head-dim on partitions)
#   v           [T part, NH] bf16 with an appended ones column per head (the
#               softmax denominator falls out of the attn matmul for free)
#   scores^T    [T part, F free] fp32 psum = kT_chunk.T @ qT_chunk (K=64 pairs
#               on row groups 0-63 / 64-127)
#   softmax     exp on ACT (psum -> bf16), then multiply by exp(bias)^T
#               (precomputed on host, bf16) on DVE in 2x bf16 mode.
#               No max-subtraction needed: |logits| <~ 12.
#   attn        accumulated as [F-tile part, (h, H+1) free] fp32 psum: the
#               attn matmuls use pt (scores^T) as the STATIONARY operand and
#               v as the moving operand, so each matmul is N=65 wide with all
#               128 output partitions used -- half the PE rows of the
#               [nh part, F free] orientation.
#   normalize   per-partition: reciprocal of the denominator column then one
#               broadcast tensor_tensor per F-tile (DVE), bf16 out.
#   transpose   [f, nh] -> [nh, f] via the XBAR DMA transpose (16x128 tiles,
#               runs on the DMA engines, not PE/DVE/ACT).
#   out         attnT.T @ wo per F-tile; psum DMA'd straight to DRAM.

import numpy as np

B, F, T, C = 2, 2048, 2048, 1024
HEADS, DEPTH = 16, 64
N_CORES = 8
HG = 4  # head-groups; heads per group = HEADS // HG = 4
# fp8e4m3 DoubleRow score matmuls: halves score-matmul PE time but raises
# rel err from ~4e-3 to ~1.7e-2 (gate is 2e-2). Off = safe margin.
USE_FP8_SCORES = False


def build_attention_nc(C=1024, F=2048, T=2048, NHEADS=4, H=64, fc_w=512,
                       use_fp8=False, debug_taps=False):
    import concourse.tile as tile
    import concourse.mybir as mybir
    from concourse import bacc

    P = 128
    NH = NHEADS * H          # local heads * depth (256)
    KC = C // P              # contraction subtiles for projections (8)
    NFC = F // fc_w          # F chunks (4)
    NTT = T // P             # T tiles (16)
    NHC = NH // P            # NH chunks of 128 partitions (2)
    FPC = fc_w // P          # F tiles per F chunk (4)
    assert NHC * 2 == NHEADS and H == 64, "layout assumes 2 heads per NH chunk"
    f32 = mybir.dt.float32
    bf16 = mybir.dt.bfloat16
    fp8 = mybir.dt.float8e4
    DR = mybir.MatmulPerfMode.DoubleRow
    Exp = mybir.ActivationFunctionType.Exp
    Mult = mybir.AluOpType.mult

    nc = bacc.Bacc("TRN2", target_bir_lowering=False, debug=False, name="attn69")

    qT_d = nc.dram_tensor("qT", [C, F], bf16, kind="ExternalInput")
    sT_d = nc.dram_tensor("sT", [C, T], bf16, kind="ExternalInput")
    eb_d = nc.dram_tensor("ebT", [T, F], bf16, kind="ExternalInput")
    id_d = nc.dram_tensor("ident", [P, P], bf16, kind="ExternalInput")
    wq_d = nc.dram_tensor("wq", [C, NH], bf16, kind="ExternalInput")
    wk_d = nc.dram_tensor("wk", [C, NH], bf16, kind="ExternalInput")
    wv_d = nc.dram_tensor("wv", [C, NH], bf16, kind="ExternalInput")
    wo_d = nc.dram_tensor("wo", [NH, C], bf16, kind="ExternalInput")
    out_d = nc.dram_tensor("out_p", [F, C], f32, kind="ExternalOutput")

    with tile.TileContext(nc) as tc:
        with (
            tc.tile_pool(name="constp", bufs=1) as constp,
            tc.tile_pool(name="persist", bufs=1) as persist,
            tc.tile_pool(name="actp", bufs=4) as actp,
            tc.tile_pool(name="biasp", bufs=6) as biasp,
            tc.tile_pool(name="ptp", bufs=3) as ptp,
            tc.tile_pool(name="smallp", bufs=4) as smallp,
            tc.tile_pool(name="normp", bufs=6) as normp,
            tc.tile_pool(name="psA", bufs=4, space="PSUM") as psA,
            tc.tile_pool(name="psS", bufs=2, space="PSUM") as psS,
        ):
            # ---------------- weights (tiles only; DMAs ordered below) -------
            wq_sb = constp.tile([P, KC, NH], bf16, name="wq_sb")
            wk_sb = constp.tile([P, KC, NH], bf16, name="wk_sb")
            wv_sb = constp.tile([P, KC, NH], bf16, name="wv_sb")
            wo_sb = constp.tile([P, NHC, C], bf16, name="wo_sb")

            # ---------------- persistent activations ----------------
            # fp8 mode: q/k live in fp8e4m3 with the DoubleRow layout --
            # partition 32*h + p holds head h's contraction rows p and 32+p
            # (kt the second dim); wq/wk columns are pre-permuted on the host
            # so the projection psum lands in exactly this partition order.
            # bf16 mode: q/k live as [head-dim part, pair, F] like v.
            qk_dt = fp8 if use_fp8 else bf16
            qT8 = persist.tile([P, 2, F], qk_dt, name="qT8")
            kT8 = persist.tile([P, 2, T], qk_dt, name="kT8")
            v_sb = persist.tile([P, NTT, NHEADS, H + 1], bf16, name="v_sb")
            attnT_sb = persist.tile([P, NHC, F], bf16, name="attnT_sb")
            # ones column for the softmax denominator (cols 0..H-1 overwritten
            # by the v projection; only col H needs initializing)
            ones1 = nc.const_aps.aps[(f32, 1.0)]
            nc.scalar.copy(
                v_sb[:, :, :, H:H + 1],
                ones1[:, None, None, :].to_broadcast((P, NTT, NHEADS, 1)))

            # ---------------- q projection (emitted per F chunk) ----------------
            # depth**-0.5 is folded into wq on the host, so this is a plain
            # psum->sbuf copy (DVE, keeping ACT free for the exps).
            qT_r = qT_d.ap().rearrange("(ko p) f -> p ko f", p=P)
            sT_r = sT_d.ap().rearrange("(ko p) t -> p ko t", p=P)
            t_per_chunk = fc_w // P

            def load_q(fc):
                qa = actp.tile([P, KC, fc_w], bf16, tag="act", name="qa")
                nc.sync.dma_start(qa[:], qT_r[:, :, fc * fc_w:(fc + 1) * fc_w])
                return qa

            def load_s(sc):
                sa = actp.tile([P, KC, fc_w], bf16, tag="act", name="sa")
                nc.sync.dma_start(sa[:], sT_r[:, :, sc * fc_w:(sc + 1) * fc_w])
                return sa

            def q_proj(fc, qa=None):
                if qa is None:
                    qa = load_q(fc)
                for m in range(NHC):
                    psq = psA.tile([P, 512], f32, tag="bank", name="psq")
                    for k in range(KC):
                        nc.tensor.matmul(
                            psq[:, :fc_w],
                            lhsT=wq_sb[:, k, m * P:(m + 1) * P],
                            rhs=qa[:, k, :],
                            start=(k == 0), stop=(k == KC - 1))
                    nc.vector.tensor_copy(
                        qT8[:, m, fc * fc_w:(fc + 1) * fc_w], psq[:, :fc_w])

            # ---------------- k and v projections ----------------
            def k_proj(sc, sa):
                for m in range(NHC):
                    psk = psA.tile([P, 512], f32, tag="bank", name="psk")
                    for k in range(KC):
                        nc.tensor.matmul(
                            psk[:, :fc_w],
                            lhsT=wk_sb[:, k, m * P:(m + 1) * P],
                            rhs=sa[:, k, :],
                            start=(k == 0), stop=(k == KC - 1))
                    nc.vector.tensor_copy(kT8[:, m, sc * fc_w:(sc + 1) * fc_w], psk[:, :fc_w])

            def v_proj(sc, sa):
                for tl in range(t_per_chunk):
                    tt = sc * t_per_chunk + tl
                    psv = psA.tile([P, 512], f32, tag="bank", name="psv")
                    for k in range(KC):
                        nc.tensor.matmul(
                            psv[:, :NH],
                            lhsT=sa[:, k, tl * P:(tl + 1) * P],
                            rhs=wv_sb[:, k, :],
                            start=(k == 0), stop=(k == KC - 1))
                    nc.vector.tensor_copy(
                        v_sb[:, tt, :, 0:H],
                        psv[:, :NH].rearrange("p (h x) -> p h x", h=NHEADS))

            # ------------- attention main loop (software-pipelined) -------------
            # Chunk fc's softmax stream (ST matmuls -> exp -> *exp(bias))
            # produces NTT pt tiles; chunk fc-1's attention accumulation,
            # normalize, and output projection are interleaved with it. Chunk
            # 0's stream overlaps the k/v projection prefix, so ACT/DVE are
            # busy during the PE-dense projection phase and across chunk
            # boundaries.
            pt_store = {}
            bias_store = {}
            eb_r = eb_d.ap().rearrange("(tg p) f -> p tg f", p=P)

            def load_b4(fc, q):
                # one bias DMA per quarter chunk (four t-tiles)
                b4 = biasp.tile([P, 4, fc_w], bf16, tag="bias", name="b4")
                nc.sync.dma_start(
                    b4[:], eb_r[:, 4 * q:4 * q + 4, fc * fc_w:(fc + 1) * fc_w])
                bias_store[(fc, q)] = b4

            def produce(fc, tt):
                fsl = slice(fc * fc_w, (fc + 1) * fc_w)
                if (fc, tt // 4) not in bias_store:
                    load_b4(fc, tt // 4)
                bias_t = bias_store[(fc, tt // 4)][:, tt % 4, :]
                if tt % 4 == 3:
                    del bias_store[(fc, tt // 4)]
                pt4 = ptp.tile([P, NHEADS, fc_w], bf16, tag="pt", bufs=18, name="pt4")
                for pair in range(NHC):
                    st2 = psS.tile([P, 2, 512], f32, tag="st", name="st2")
                    for j in range(2):
                        h = 2 * pair + j
                        if use_fp8:
                            nc.tensor.matmul(
                                st2[:, j, :fc_w],
                                lhsT=kT8[32 * h:32 * h + 32, :, tt * P:(tt + 1) * P],
                                rhs=qT8[32 * h:32 * h + 32, :, fsl],
                                start=True, stop=True, perf_mode=DR,
                                tile_position=(32 * h, 0))
                        else:
                            off = j * H
                            nc.tensor.matmul(
                                st2[:, j, :fc_w],
                                lhsT=kT8[off:off + H, pair, tt * P:(tt + 1) * P],
                                rhs=qT8[off:off + H, pair, fsl],
                                start=True, stop=True)
                    # exp(S^T), psum fp32 -> bf16
                    nc.scalar.activation(
                        pt4[:, 2 * pair:2 * pair + 2, :], st2[:, :, :fc_w], Exp)
                # multiply by exp(bias)^T tile, bf16 2x mode; bias is
                # broadcast over the head dim (stride-0): one instr per tile
                nc.vector.tensor_mul(
                    pt4[:], pt4[:],
                    bias_t[:, None, :].to_broadcast((P, NHEADS, fc_w)))
                pt_store[(fc, tt)] = pt4

            def consume(fc, tt, at_tiles):
                # at_tiles[ft] accumulates [128 f, (h, H+1)] for F-tile ft;
                # pt is the stationary operand so each matmul is only N=65.
                # The bank is pre-zeroed by alloc_at and every matmul runs
                # with start=False: a start=True in a bank discards any other
                # open accumulation group's partials there (measured on HW),
                # so per-head start flags cannot share a bank.
                pt4 = pt_store.pop((fc, tt))
                for ft in range(FPC):
                    at = at_tiles[ft]
                    for h in range(NHEADS):
                        nc.tensor.matmul(
                            at[:, h, :],
                            lhsT=pt4[:, h, ft * P:(ft + 1) * P],
                            rhs=v_sb[:, tt, h, :],
                            start=False, stop=(tt == NTT - 1),
                            skip_group_check=True)

            cw = min(512, C)

            def norm_ft(fc, ft, at):
                # normalize: attn[f, h, :] = acc[f, h, 0:H] / acc[f, h, H]
                rec = smallp.tile([P, NHEADS], f32, tag="rec", name="rec")
                nc.vector.reciprocal(rec[:], at[:, :, H])
                an = normp.tile([P, NHEADS, H], bf16, tag="an", name="an")
                nc.vector.tensor_tensor(
                    an[:], at[:, :, 0:H],
                    rec[:, :, None].to_broadcast((P, NHEADS, H)), Mult)
                return an

            def tail_ft(fc, ft, an):
                # XBAR-DMA-transpose the [128 f, 128 nh] blocks into attnT_sb,
                # then this F-tile's output projection
                ftA = fc * FPC + ft
                for m in range(NHC):
                    nc.sync.dma_start_transpose(
                        attnT_sb[:, m, ftA * P:(ftA + 1) * P],
                        an[:, 2 * m:2 * m + 2, :])
                for cc in range(C // cw):
                    pso = psA.tile([P, 512], f32, tag="bank", name="pso")
                    for m in range(NHC):
                        nc.tensor.matmul(
                            pso[:, :cw],
                            lhsT=attnT_sb[:, m, ftA * P:(ftA + 1) * P],
                            rhs=wo_sb[:, m, cc * cw:(cc + 1) * cw],
                            start=(m == 0), stop=(m == NHC - 1))
                    ot = normp.tile([P, 512], f32, tag="o", name="ot")
                    nc.vector.tensor_copy(ot[:, :cw], pso[:, :cw])
                    nc.sync.dma_start(
                        out_d.ap()[ftA * P:(ftA + 1) * P, cc * cw:(cc + 1) * cw],
                        ot[:, :cw])

            def make_piece(fc, ft, cc, an):
                # one drip-feedable slice of finish: the F-tile's transposes
                # (first slice only) plus one outproj psum + copy + store
                def piece():
                    ftA = fc * FPC + ft
                    if cc == 0:
                        for m in range(NHC):
                            nc.sync.dma_start_transpose(
                                attnT_sb[:, m, ftA * P:(ftA + 1) * P],
                                an[:, 2 * m:2 * m + 2, :])
                    pso = psA.tile([P, 512], f32, tag="bank", name="pso")
                    for m in range(NHC):
                        nc.tensor.matmul(
                            pso[:, :cw],
                            lhsT=attnT_sb[:, m, ftA * P:(ftA + 1) * P],
                            rhs=wo_sb[:, m, cc * cw:(cc + 1) * cw],
                            start=(m == 0), stop=(m == NHC - 1))
                    ot = normp.tile([P, 512], f32, tag="o", bufs=6, name="ot")
                    nc.vector.tensor_copy(ot[:, :cw], pso[:, :cw])
                    nc.sync.dma_start(
                        out_d.ap()[ftA * P:(ftA + 1) * P, cc * cw:(cc + 1) * cw],
                        ot[:, :cw])
                return piece

            # PE p-state warm-up: ~45 dummy matmuls on scratch data keep the
            # PE continuously busy from t~0.7us so the first real projection
            # matmuls run at the full 2.4 GHz clock instead of 0.65/1.2 GHz.
            scr = smallp.tile([P, 512], bf16, tag="scr", name="scr")
            nc.gpsimd.memset(scr[:], 0.0)
            pswu = psA.tile([P, 512], f32, tag="bank", name="pswu")
            for _ in range(45):
                nc.tensor.matmul(
                    pswu[:, :128], lhsT=scr[:, 0:128], rhs=scr[:, 128:256],
                    start=True, stop=True, skip_group_check=True)

            # chunk 0 production rides along with the k/v projections.
            # DMA issue order is tuned for the serial DMA pipe: k-proj inputs
            # first (sa half, wk), then q (qa, wq), then the first bias
            # quarter and wv; wo (needed ~60us in) goes last.
            sa0 = actp.tile([P, KC, fc_w], bf16, tag="act", name="sa0")
            nc.sync.dma_start(sa0[:, 0:KC // 2, :], sT_r[:, 0:KC // 2, 0:fc_w])
            nc.sync.dma_start(
                wk_sb[:], wk_d.ap().rearrange("(ko p) m -> p ko m", p=P))
            nc.sync.dma_start(sa0[:, KC // 2:, :], sT_r[:, KC // 2:, 0:fc_w])
            qa0 = actp.tile([P, KC, fc_w], bf16, tag="act", name="qa0")
            nc.sync.dma_start(qa0[:, 0:KC // 2, :], qT_r[:, 0:KC // 2, 0:fc_w])
            nc.sync.dma_start(
                wq_sb[:], wq_d.ap().rearrange("(ko p) m -> p ko m", p=P))
            nc.sync.dma_start(qa0[:, KC // 2:, :], qT_r[:, KC // 2:, 0:fc_w])
            load_b4(0, 0)
            nc.sync.dma_start(
                wv_sb[:], wv_d.ap().rearrange("(ko p) m -> p ko m", p=P))
            nc.sync.dma_start(
                wo_sb[:], wo_d.ap().rearrange("(ko p) m -> p ko m", p=P))
            id_sb = constp.tile([P, P], bf16, name="id_sb")
            nc.sync.dma_start(id_sb[:], id_d.ap())

            # emission order puts each source chunk's k-projection and the
            # score/exp stream before its v-projection (v is only needed by
            # the much-later consume stage)
            sa_next = None
            for sc in range(T // fc_w):
                sa = sa0 if sc == 0 else sa_next
                k_proj(sc, sa)
                if sc == 0:
                    q_proj(0, qa0)
                if sc + 1 < T // fc_w:
                    load_b4(0, sc + 1)
                    sa_next = load_s(sc + 1)
                for tl in range(t_per_chunk):
                    produce(0, sc * t_per_chunk + tl)
                v_proj(sc, sa)
            for fc in range(1, NFC):
                q_proj(fc)

            def alloc_at():
                tiles = []
                for ft in range(FPC):
                    at = psA.tile([P, NHEADS, H + 1], f32, tag="bank", name=f"at{ft}")
                    nc.vector.memset(at[:], 0.0)
                    tiles.append(at)
                return tiles

            # Greedy catch-up pipeline: the produce stream runs continuously
            # (chunks 1..NFC-1) while consumes trail, draining at up to two
            # units per produce slot. A unit is either a consume tile or a
            # finish "piece" (transpose + one outproj psum): finish work is
            # drip-fed across slots so it never bursts the PE at a chunk
            # boundary, and piece psums recycle the accumulator banks BEFORE
            # the next chunk's memsets claim them (pool request order).
            at_cur = alloc_at()  # chunk 0 accumulators
            c = 0          # global consume pointer (tile index)
            produced = NTT  # chunk 0 fully produced in the prefix
            pending = []   # queued finish pieces
            need_alloc = False

            def emit_slot_work(budget):
                nonlocal c, at_cur, need_alloc
                while budget > 0:
                    if pending:
                        pending.pop(0)()
                        budget -= 1
                        continue
                    if need_alloc:
                        at_cur = alloc_at()
                        need_alloc = False
                    if at_cur is None or c >= produced - 1:
                        break
                    cfc, ctt = divmod(c, NTT)
                    consume(cfc, ctt, at_cur)
                    c += 1
                    budget -= 1
                    if ctt == NTT - 1:
                        ans = [norm_ft(cfc, ft, at_cur[ft]) for ft in range(FPC)]
                        for ft in range(FPC):
                            for cc in range(C // cw):
                                pending.append(make_piece(cfc, ft, cc, ans[ft]))
                        at_cur = None
                        need_alloc = cfc + 1 < NFC

            for fc in range(1, NFC):
                for tt in range(NTT):
                    produce(fc, tt)
                    produced += 1
                    emit_slot_work(2)
            # Tail: flush pending pieces and stragglers, then drain the last
            # chunk F-tile-major so each F-tile's normalize/transpose/outproj
            # overlaps the next F-tile's accumulation matmuls.
            while pending:
                pending.pop(0)()
            if need_alloc:
                at_cur = alloc_at()
                need_alloc = False
            while c < (NFC - 1) * NTT:
                cfc, ctt = divmod(c, NTT)
                consume(cfc, ctt, at_cur)
                c += 1
                if ctt == NTT - 1:
                    ans = [norm_ft(cfc, ft, at_cur[ft]) for ft in range(FPC)]
                    for ft in range(FPC):
                        tail_ft(cfc, ft, ans[ft])
                    at_cur = alloc_at()
            last = NFC - 1
            rem = [divmod(g, NTT)[1] for g in range(c, NFC * NTT)]
            pts = {tt: pt_store.pop((last, tt)) for tt in rem}
            for ft in range(FPC):
                at = at_cur[ft]
                for tt in rem:
                    for h in range(NHEADS):
                        nc.tensor.matmul(
                            at[:, h, :],
                            lhsT=pts[tt][:, h, ft * P:(ft + 1) * P],
                            rhs=v_sb[:, tt, h, :],
                            start=False, stop=(tt == rem[-1]),
                            skip_group_check=True)
                an = norm_ft(last, ft, at)
                # fast tail: PE transposes (identity matmul) + ACT copies
                # keep the last chunk's critical chain off the serial
                # HWDGE/DMA pipe; output DMA batched to one per F-tile
                ftA = last * FPC + ft
                pst = psA.tile([P, 512], f32, tag="bank", name="pst")
                pst_b = pst[:].bitcast(bf16)
                for m in range(NHC):
                    nc.tensor.matmul(
                        pst_b[:, m * P:(m + 1) * P],
                        lhsT=an[:, 2 * m:2 * m + 2, :],
                        rhs=id_sb[:],
                        start=True, stop=True, is_transpose=True,
                        skip_group_check=True)
                    nc.scalar.copy(
                        attnT_sb[:, m, ftA * P:(ftA + 1) * P],
                        pst_b[:, m * P:(m + 1) * P])
                ot = normp.tile([P, 2, 512], f32, tag="o2", bufs=2, name="ot2")
                for cc in range(C // cw):
                    pso = psA.tile([P, 512], f32, tag="bank", name="pso")
                    for m in range(NHC):
                        nc.tensor.matmul(
                            pso[:, :cw],
                            lhsT=attnT_sb[:, m, ftA * P:(ftA + 1) * P],
                            rhs=wo_sb[:, m, cc * cw:(cc + 1) * cw],
                            start=(m == 0), stop=(m == NHC - 1))
                    nc.vector.tensor_copy(ot[:, cc, :cw], pso[:, :cw])
                nc.sync.dma_start(
                    out_d.ap()[ftA * P:(ftA + 1) * P, :], ot[:])

    nc.compile()
    return nc


_CACHE = {}


def _get_nc():
    if "nc" not in _CACHE:
        _CACHE["nc"] = build_attention_nc(C=C, F=F, T=T, NHEADS=HEADS // HG,
                                          H=DEPTH, use_fp8=USE_FP8_SCORES)
    return _CACHE["nc"]


def kernel(query_input, source_input, bias, wq, wk, wv, wo, **run_kwargs):
    import ml_dtypes
    from concourse.bass_utils import run_bass_kernel_spmd

    bf = ml_dtypes.bfloat16
    q = np.asarray(query_input, dtype=np.float32)
    s = np.asarray(source_input, dtype=np.float32)
    b = np.asarray(bias, dtype=np.float32)
    scale = float(DEPTH) ** -0.5
    wq2 = np.asarray(wq, dtype=np.float32).reshape(C, HEADS * DEPTH) * scale
    wk2 = np.asarray(wk, dtype=np.float32).reshape(C, HEADS * DEPTH)
    wv2 = np.asarray(wv, dtype=np.float32).reshape(C, HEADS * DEPTH)
    wo2 = np.asarray(wo, dtype=np.float32).reshape(HEADS * DEPTH, C)

    qT = [np.ascontiguousarray(q[i].T).astype(bf) for i in range(B)]
    sT = [np.ascontiguousarray(s[i].T).astype(bf) for i in range(B)]
    ebT = np.exp(np.ascontiguousarray(b[0, 0].T)).astype(bf)

    nhl = (HEADS // HG) * DEPTH  # NH columns per core (256)
    # DoubleRow column permutation: psum partition pi of matmul group m must
    # hold nh = (pi//32)*64 + 32*m + pi%32 (head pi//32, k-tile m, row pi%32)
    pi = np.arange(128)
    if USE_FP8_SCORES:
        dr_perm = np.concatenate([(pi // 32) * 64 + 32 * m + pi % 32 for m in (0, 1)])
    else:
        dr_perm = np.arange(2 * 128)
    in_maps = []
    for c in range(N_CORES):
        bi, hg = c // HG, c % HG
        sl = slice(hg * nhl, (hg + 1) * nhl)
        in_maps.append({
            "qT": qT[bi],
            "sT": sT[bi],
            "ebT": ebT,
            "ident": np.eye(128, dtype=np.float32).astype(bf),
            "wq": np.ascontiguousarray(wq2[:, sl][:, dr_perm]).astype(bf),
            "wk": np.ascontiguousarray(wk2[:, sl][:, dr_perm]).astype(bf),
            "wv": np.ascontiguousarray(wv2[:, sl]).astype(bf),
            "wo": np.ascontiguousarray(wo2[sl, :]).astype(bf),
        })

    nc = _get_nc()
    res = run_bass_kernel_spmd(nc, in_maps, core_ids=list(range(N_CORES)), **run_kwargs)
    _CACHE["last_results"] = res

    out = np.empty((B, F, C), np.float32)
    for bi in range(B):
        acc = res.results[bi * HG]["out_p"].astype(np.float32)
        for hg in range(1, HG):
            acc = acc + res.results[bi * HG + hg]["out_p"]
        out[bi] = acc
    return out


# revision 47
# speedup vs baseline: 1.0559x; 1.0296x over previous
# Trainium2 Bass kernel for nn_AttentionLayer_69380901699611.
#
# Full-input contract: kernel(**inputs) takes the unsharded numpy inputs and
# returns the full [B, F, HIDDEN] output. Internally the work is sharded over
# 8 NeuronCores as (batch x head-group): core c handles batch c//4 and heads
# [4*(c%4), 4*(c%4)+4). Each core computes a partial output projection over
# its 4 heads; the host sums the 4 partials per batch.
#
# Device kernel layout (per core):
#   qT, kT      [NH=256 part-chunks, F/T] bf16 (head-dim on partitions)
#   v           [T part, NH] bf16 with an appended ones column per head (the
#               softmax denominator falls out of the attn matmul for free)
#   scores^T    [T part, F free] fp32 psum = kT_chunk.T @ qT_chunk (K=64 pairs
#               on row groups 0-63 / 64-127)
#   softmax     exp on ACT (psum -> bf16), then multiply by exp(bias)^T
#               (precomputed on host, bf16) on DVE in 2x bf16 mode.
#               No max-subtraction needed: |logits| <~ 12.
#   attn        accumulated as [F-tile part, (h, H+1) free] fp32 psum: the
#               attn matmuls use pt (scores^T) as the STATIONARY operand and
#               v as the moving operand, so each matmul is N=65 wide with all
#               128 output partitions used -- half the PE rows of the
#               [nh part, F free] orientation.
#   normalize   per-partition: reciprocal of the denominator column then one
#               broadcast tensor_tensor per F-tile (DVE), bf16 out.
#   transpose   [f, nh] -> [nh, f] via the XBAR DMA transpose (16x128 tiles,
#               runs on the DMA engines, not PE/DVE/ACT).
#   out         attnT.T @ wo per F-tile; psum DMA'd straight to DRAM.

import numpy as np

B, F, T, C = 2, 2048, 2048, 1024
HEADS, DEPTH = 16, 64
N_CORES = 8
HG = 4  # head-groups; heads per group = HEADS // HG = 4
# fp8e4m3 DoubleRow score matmuls: halves score-matmul PE time but raises
# rel err from ~4e-3 to ~1.7e-2 (gate is 2e-2). Off = safe margin.
USE_FP8_SCORES = False


def build_attention_nc(C=1024, F=2048, T=2048, NHEADS=4, H=64, fc_w=512,
                       use_fp8=False, debug_taps=False):
    import concourse.tile as tile
    import concourse.mybir as mybir
    from concourse import bacc

    P = 128
    NH = NHEADS * H          # local heads * depth (256)
    KC = C // P              # contraction subtiles for projections (8)
    NFC = F // fc_w          # F chunks (4)
    NTT = T // P             # T tiles (16)
    NHC = NH // P            # NH chunks of 128 partitions (2)
    FPC = fc_w // P          # F tiles per F chunk (4)
    assert NHC * 2 == NHEADS and H == 64, "layout assumes 2 heads per NH chunk"
    f32 = mybir.dt.float32
    bf16 = mybir.dt.bfloat16
    fp8 = mybir.dt.float8e4
    DR = mybir.MatmulPerfMode.DoubleRow
    Exp = mybir.ActivationFunctionType.Exp
    Mult = mybir.AluOpType.mult

    nc = bacc.Bacc("TRN2", target_bir_lowering=False, debug=False, name="attn69")

    qT_d = nc.dram_tensor("qT", [C, F], bf16, kind="ExternalInput")
    sT_d = nc.dram_tensor("sT", [C, T], bf16, kind="ExternalInput")
    eb_d = nc.dram_tensor("ebT", [T, F], bf16, kind="ExternalInput")
    id_d = nc.dram_tensor("ident", [P, P], bf16, kind="ExternalInput")
    wq_d = nc.dram_tensor("wq", [C, NH], bf16, kind="ExternalInput")
    wk_d = nc.dram_tensor("wk", [C, NH], bf16, kind="ExternalInput")
    wv_d = nc.dram_tensor("wv", [C, NH], bf16, kind="ExternalInput")
    wo_d = nc.dram_tensor("wo", [NH, C], bf16, kind="ExternalInput")
    out_d = nc.dram_tensor("out_p", [F, C], f32, kind="ExternalOutput")

    with tile.TileContext(nc) as tc:
        with (
            tc.tile_pool(name="constp", bufs=1) as constp,
            tc.tile_pool(name="persist", bufs=1) as persist,
            tc.tile_pool(name="actp", bufs=4) as actp,
            tc.tile_pool(name="biasp", bufs=6) as biasp,
            tc.tile_pool(name="ptp", bufs=3) as ptp,
            tc.tile_pool(name="smallp", bufs=4) as smallp,
            tc.tile_pool(name="normp", bufs=6) as normp,
            tc.tile_pool(name="psA", bufs=4, space="PSUM") as psA,
            tc.tile_pool(name="psS", bufs=2, space="PSUM") as psS,
        ):
            # ---------------- weights (tiles only; DMAs ordered below) -------
            wq_sb = constp.tile([P, KC, NH], bf16, name="wq_sb")
            wk_sb = constp.tile([P, KC, NH], bf16, name="wk_sb")
            wv_sb = constp.tile([P, KC, NH], bf16, name="wv_sb")
            wo_sb = constp.tile([P, NHC, C], bf16, name="wo_sb")

            # ---------------- persistent activations ----------------
            # fp8 mode: q/k live in fp8e4m3 with the DoubleRow layout --
            # partition 32*h + p holds head h's contraction rows p and 32+p
            # (kt the second dim); wq/wk columns are pre-permuted on the host
            # so the projection psum lands in exactly this partition order.
            # bf16 mode: q/k live as [head-dim part, pair, F] like v.
            qk_dt = fp8 if use_fp8 else bf16
            qT8 = persist.tile([P, 2, F], qk_dt, name="qT8")
            kT8 = persist.tile([P, 2, T], qk_dt, name="kT8")
            v_sb = persist.tile([P, NTT, NHEADS, H + 1], bf16, name="v_sb")
            attnT_sb = persist.tile([P, NHC, F], bf16, name="attnT_sb")
            # ones column for the softmax denominator (cols 0..H-1 overwritten
            # by the v projection; only col H needs initializing)
            ones1 = nc.const_aps.aps[(f32, 1.0)]
            nc.scalar.copy(
                v_sb[:, :, :, H:H + 1],
                ones1[:, None, None, :].to_broadcast((P, NTT, NHEADS, 1)))

            # ---------------- q projection (emitted per F chunk) ----------------
            # depth**-0.5 is folded into wq on the host, so this is a plain
            # psum->sbuf copy (DVE, keeping ACT free for the exps).
            qT_r = qT_d.ap().rearrange("(ko p) f -> p ko f", p=P)
            sT_r = sT_d.ap().rearrange("(ko p) t -> p ko t", p=P)
            t_per_chunk = fc_w // P

            def load_q(fc):
                qa = actp.tile([P, KC, fc_w], bf16, tag="act", name="qa")
                nc.sync.dma_start(qa[:], qT_r[:, :, fc * fc_w:(fc + 1) * fc_w])
                return qa

            def load_s(sc):
                sa = actp.tile([P, KC, fc_w], bf16, tag="act", name="sa")
                nc.sync.dma_start(sa[:], sT_r[:, :, sc * fc_w:(sc + 1) * fc_w])
                return sa

            def q_proj_m(fc, m, qa):
                psq = psA.tile([P, 512], f32, tag="bank", name="psq")
                for k in range(KC):
                    nc.tensor.matmul(
                        psq[:, :fc_w],
                        lhsT=wq_sb[:, k, m * P:(m + 1) * P],
                        rhs=qa[:, k, :],
                        start=(k == 0), stop=(k == KC - 1))
                nc.vector.tensor_copy(
                    qT8[:, m, fc * fc_w:(fc + 1) * fc_w], psq[:, :fc_w])

            def q_proj(fc, qa=None):
                if qa is None:
                    qa = load_q(fc)
                for m in range(NHC):
                    q_proj_m(fc, m, qa)

            # ---------------- k and v projections ----------------
            def k_proj(sc, sa):
                for m in range(NHC):
                    psk = psA.tile([P, 512], f32, tag="bank", name="psk")
                    for k in range(KC):
                        nc.tensor.matmul(
                            psk[:, :fc_w],
                            lhsT=wk_sb[:, k, m * P:(m + 1) * P],
                            rhs=sa[:, k, :],
                            start=(k == 0), stop=(k == KC - 1))
                    nc.vector.tensor_copy(kT8[:, m, sc * fc_w:(sc + 1) * fc_w], psk[:, :fc_w])

            def v_proj(sc, sa):
                for tl in range(t_per_chunk):
                    tt = sc * t_per_chunk + tl
                    psv = psA.tile([P, 512], f32, tag="bank", name="psv")
                    for k in range(KC):
                        nc.tensor.matmul(
                            psv[:, :NH],
                            lhsT=sa[:, k, tl * P:(tl + 1) * P],
                            rhs=wv_sb[:, k, :],
                            start=(k == 0), stop=(k == KC - 1))
                    nc.vector.tensor_copy(
                        v_sb[:, tt, :, 0:H],
                        psv[:, :NH].rearrange("p (h x) -> p h x", h=NHEADS))

            # ------------- attention main loop (software-pipelined) -------------
            # Chunk fc's softmax stream (ST matmuls -> exp -> *exp(bias))
            # produces NTT pt tiles; chunk fc-1's attention accumulation,
            # normalize, and output projection are interleaved with it. Chunk
            # 0's stream overlaps the k/v projection prefix, so ACT/DVE are
            # busy during the PE-dense projection phase and across chunk
            # boundaries.
            pt_store = {}
            bias_store = {}
            eb_r = eb_d.ap().rearrange("(tg p) f -> p tg f", p=P)

            def load_b4(fc, q):
                # one bias DMA per quarter chunk (four t-tiles)
                b4 = biasp.tile([P, 4, fc_w], bf16, tag="bias", name="b4")
                nc.sync.dma_start(
                    b4[:], eb_r[:, 4 * q:4 * q + 4, fc * fc_w:(fc + 1) * fc_w])
                bias_store[(fc, q)] = b4

            def produce(fc, tt):
                fsl = slice(fc * fc_w, (fc + 1) * fc_w)
                if (fc, tt // 4) not in bias_store:
                    load_b4(fc, tt // 4)
                bias_t = bias_store[(fc, tt // 4)][:, tt % 4, :]
                if tt % 4 == 3:
                    del bias_store[(fc, tt // 4)]
                pt4 = ptp.tile([P, NHEADS, fc_w], bf16, tag="pt", bufs=18, name="pt4")
                for pair in range(NHC):
                    st2 = psS.tile([P, 2, 512], f32, tag="st", name="st2")
                    for j in range(2):
                        h = 2 * pair + j
                        if use_fp8:
                            nc.tensor.matmul(
                                st2[:, j, :fc_w],
                                lhsT=kT8[32 * h:32 * h + 32, :, tt * P:(tt + 1) * P],
                                rhs=qT8[32 * h:32 * h + 32, :, fsl],
                                start=True, stop=True, perf_mode=DR,
                                tile_position=(32 * h, 0))
                        else:
                            off = j * H
                            nc.tensor.matmul(
                                st2[:, j, :fc_w],
                                lhsT=kT8[off:off + H, pair, tt * P:(tt + 1) * P],
                                rhs=qT8[off:off + H, pair, fsl],
                                start=True, stop=True)
                    # exp(S^T), psum fp32 -> bf16
                    nc.scalar.activation(
                        pt4[:, 2 * pair:2 * pair + 2, :], st2[:, :, :fc_w], Exp)
                # multiply by exp(bias)^T tile, bf16 2x mode; bias is
                # broadcast over the head dim (stride-0): one instr per tile
                nc.vector.tensor_mul(
                    pt4[:], pt4[:],
                    bias_t[:, None, :].to_broadcast((P, NHEADS, fc_w)))
                pt_store[(fc, tt)] = pt4

            def consume(fc, tt, at_tiles):
                # at_tiles[ft] accumulates [128 f, (h, H+1)] for F-tile ft;
                # pt is the stationary operand so each matmul is only N=65.
                # The bank is pre-zeroed by alloc_at and every matmul runs
                # with start=False: a start=True in a bank discards any other
                # open accumulation group's partials there (measured on HW),
                # so per-head start flags cannot share a bank.
                pt4 = pt_store.pop((fc, tt))
                for ft in range(FPC):
                    at = at_tiles[ft]
                    for h in range(NHEADS):
                        nc.tensor.matmul(
                            at[:, h, :],
                            lhsT=pt4[:, h, ft * P:(ft + 1) * P],
                            rhs=v_sb[:, tt, h, :],
                            start=False, stop=(tt == NTT - 1),
                            skip_group_check=True)

            cw = min(512, C)

            def norm_ft(fc, ft, at):
                # normalize: attn[f, h, :] = acc[f, h, 0:H] / acc[f, h, H]
                rec = smallp.tile([P, NHEADS], f32, tag="rec", name="rec")
                nc.vector.reciprocal(rec[:], at[:, :, H])
                an = normp.tile([P, NHEADS, H], bf16, tag="an", name="an")
                nc.vector.tensor_tensor(
                    an[:], at[:, :, 0:H],
                    rec[:, :, None].to_broadcast((P, NHEADS, H)), Mult)
                return an

            def tail_ft(fc, ft, an):
                # XBAR-DMA-transpose the [128 f, 128 nh] blocks into attnT_sb,
                # then this F-tile's output projection
                ftA = fc * FPC + ft
                for m in range(NHC):
                    nc.sync.dma_start_transpose(
                        attnT_sb[:, m, ftA * P:(ftA + 1) * P],
                        an[:, 2 * m:2 * m + 2, :])
                for cc in range(C // cw):
                    pso = psA.tile([P, 512], f32, tag="bank", name="pso")
                    for m in range(NHC):
                        nc.tensor.matmul(
                            pso[:, :cw],
                            lhsT=attnT_sb[:, m, ftA * P:(ftA + 1) * P],
                            rhs=wo_sb[:, m, cc * cw:(cc + 1) * cw],
                            start=(m == 0), stop=(m == NHC - 1))
                    ot = normp.tile([P, 512], f32, tag="o", name="ot")
                    nc.vector.tensor_copy(ot[:, :cw], pso[:, :cw])
                    nc.sync.dma_start(
                        out_d.ap()[ftA * P:(ftA + 1) * P, cc * cw:(cc + 1) * cw],
                        ot[:, :cw])

            def make_piece(fc, ft, cc, an):
                # one drip-feedable slice of finish: the F-tile's transposes
                # (first slice only) plus one outproj psum + copy + store
                def piece():
                    ftA = fc * FPC + ft
                    if cc == 0:
                        for m in range(NHC):
                            nc.sync.dma_start_transpose(
                                attnT_sb[:, m, ftA * P:(ftA + 1) * P],
                                an[:, 2 * m:2 * m + 2, :])
                    pso = psA.tile([P, 512], f32, tag="bank", name="pso")
                    for m in range(NHC):
                        nc.tensor.matmul(
                            pso[:, :cw],
                            lhsT=attnT_sb[:, m, ftA * P:(ftA + 1) * P],
                            rhs=wo_sb[:, m, cc * cw:(cc + 1) * cw],
                            start=(m == 0), stop=(m == NHC - 1))
                    ot = normp.tile([P, 512], f32, tag="o", bufs=6, name="ot")
                    nc.vector.tensor_copy(ot[:, :cw], pso[:, :cw])
                    nc.sync.dma_start(
                        out_d.ap()[ftA * P:(ftA + 1) * P, cc * cw:(cc + 1) * cw],
                        ot[:, :cw])
                return piece

            # PE p-state warm-up: ~45 dummy matmuls on scratch data keep the
            # PE continuously busy from t~0.7us so the first real projection
            # matmuls run at the full 2.4 GHz clock instead of 0.65/1.2 GHz.
            scr = smallp.tile([P, 512], bf16, tag="scr", name="scr")
            nc.gpsimd.memset(scr[:], 0.0)
            pswu = psA.tile([P, 512], f32, tag="bank", name="pswu")
            for _ in range(45):
                nc.tensor.matmul(
                    pswu[:, :128], lhsT=scr[:, 0:128], rhs=scr[:, 128:256],
                    start=True, stop=True, skip_group_check=True)

            # chunk 0 production rides along with the k/v projections.
            # DMA issue order is tuned for the serial DMA pipe: k-proj inputs
            # first (sa half, wk), then q (qa, wq), then the first bias
            # quarter and wv; wo (needed ~60us in) goes last.
            sa0 = actp.tile([P, KC, fc_w], bf16, tag="act", name="sa0")
            nc.sync.dma_start(sa0[:, 0:KC // 2, :], sT_r[:, 0:KC // 2, 0:fc_w])
            nc.sync.dma_start(
                wk_sb[:], wk_d.ap().rearrange("(ko p) m -> p ko m", p=P))
            nc.sync.dma_start(sa0[:, KC // 2:, :], sT_r[:, KC // 2:, 0:fc_w])
            qa0 = actp.tile([P, KC, fc_w], bf16, tag="act", name="qa0")
            nc.sync.dma_start(qa0[:, 0:KC // 2, :], qT_r[:, 0:KC // 2, 0:fc_w])
            nc.sync.dma_start(
                wq_sb[:], wq_d.ap().rearrange("(ko p) m -> p ko m", p=P))
            nc.sync.dma_start(qa0[:, KC // 2:, :], qT_r[:, KC // 2:, 0:fc_w])
            load_b4(0, 0)
            nc.sync.dma_start(
                wv_sb[:], wv_d.ap().rearrange("(ko p) m -> p ko m", p=P))
            nc.sync.dma_start(
                wo_sb[:], wo_d.ap().rearrange("(ko p) m -> p ko m", p=P))
            id_sb = constp.tile([P, P], bf16, name="id_sb")
            nc.sync.dma_start(id_sb[:], id_d.ap())

            # emission order puts each source chunk's k-projection and the
            # score/exp stream before its v-projection (v is only needed by
            # the much-later consume stage)
            sa_next = None
            for sc in range(T // fc_w):
                sa = sa0 if sc == 0 else sa_next
                k_proj(sc, sa)
                if sc == 0:
                    q_proj(0, qa0)
                if sc + 1 < T // fc_w:
                    load_b4(0, sc + 1)
                    sa_next = load_s(sc + 1)
                for tl in range(t_per_chunk):
                    produce(0, sc * t_per_chunk + tl)
                v_proj(sc, sa)
            # q(1) is needed immediately by the main loop; q(2)/q(3) are
            # dripped into the first main-loop slots (one m-group per slot)
            q_proj(1)
            deferred_q = []
            for fc in (2, 3):
                qa = load_q(fc)
                for m in range(NHC):
                    deferred_q.append((fc, m, qa))

            def alloc_at():
                tiles = []
                for ft in range(FPC):
                    at = psA.tile([P, NHEADS, H + 1], f32, tag="bank", name=f"at{ft}")
                    nc.vector.memset(at[:], 0.0)
                    tiles.append(at)
                return tiles

            # Greedy catch-up pipeline: the produce stream runs continuously
            # (chunks 1..NFC-1) while consumes trail, draining at up to two
            # units per produce slot. A unit is either a consume tile or a
            # finish "piece" (transpose + one outproj psum): finish work is
            # drip-fed across slots so it never bursts the PE at a chunk
            # boundary, and piece psums recycle the accumulator banks BEFORE
            # the next chunk's memsets claim them (pool request order).
            at_cur = alloc_at()  # chunk 0 accumulators
            c = 0          # global consume pointer (tile index)
            produced = NTT  # chunk 0 fully produced in the prefix
            pending = []   # queued finish pieces
            need_alloc = False

            def emit_slot_work(budget):
                nonlocal c, at_cur, need_alloc
                while budget > 0:
                    if pending:
                        pending.pop(0)()
                        budget -= 1
                        continue
                    if need_alloc:
                        at_cur = alloc_at()
                        need_alloc = False
                    if at_cur is None or c >= produced - 1:
                        break
                    cfc, ctt = divmod(c, NTT)
                    consume(cfc, ctt, at_cur)
                    c += 1
                    budget -= 1
                    if ctt == NTT - 1:
                        ans = [norm_ft(cfc, ft, at_cur[ft]) for ft in range(FPC)]
                        for ft in range(FPC):
                            for cc in range(C // cw):
                                pending.append(make_piece(cfc, ft, cc, ans[ft]))
                        at_cur = None
                        need_alloc = cfc + 1 < NFC

            for fc in range(1, NFC):
                for tt in range(NTT):
                    produce(fc, tt)
                    produced += 1
                    if deferred_q:
                        q_proj_m(*deferred_q.pop(0))
                        emit_slot_work(1)
                    else:
                        emit_slot_work(2)
            # Tail: flush pending pieces and stragglers, then drain the last
            # chunk F-tile-major so each F-tile's normalize/transpose/outproj
            # overlaps the next F-tile's accumulation matmuls.
            while pending:
                pending.pop(0)()
            if need_alloc:
                at_cur = alloc_at()
                need_alloc = False
            while c < (NFC - 1) * NTT:
                cfc, ctt = divmod(c, NTT)
                consume(cfc, ctt, at_cur)
                c += 1
                if ctt == NTT - 1:
                    ans = [norm_ft(cfc, ft, at_cur[ft]) for ft in range(FPC)]
                    for ft in range(FPC):
                        tail_ft(cfc, ft, ans[ft])
                    at_cur = alloc_at()
            last = NFC - 1
            rem = [divmod(g, NTT)[1] for g in range(c, NFC * NTT)]
            pts = {tt: pt_store.pop((last, tt)) for tt in rem}
            for ft in range(FPC):
                at = at_cur[ft]
                for tt in rem:
                    for h in range(NHEADS):
                        nc.tensor.matmul(
                            at[:, h, :],
                            lhsT=pts[tt][:, h, ft * P:(ft + 1) * P],
                            rhs=v_sb[:, tt, h, :],
                            start=False, stop=(tt == rem[-1]),
                            skip_group_check=True)
                an = norm_ft(last, ft, at)
                # fast tail: PE transposes (identity matmul) + ACT copies
                # keep the last chunk's critical chain off the serial
                # HWDGE/DMA pipe; output DMA batched to one per F-tile
                ftA = last * FPC + ft
                pst = psA.tile([P, 512], f32, tag="bank", name="pst")
                pst_b = pst[:].bitcast(bf16)
                for m in range(NHC):
                    nc.tensor.matmul(
                        pst_b[:, m * P:(m + 1) * P],
                        lhsT=an[:, 2 * m:2 * m + 2, :],
                        rhs=id_sb[:],
                        start=True, stop=True, is_transpose=True,
                        skip_group_check=True)
                    nc.scalar.copy(
                        attnT_sb[:, m, ftA * P:(ftA + 1) * P],
                        pst_b[:, m * P:(m + 1) * P])
                ot = normp.tile([P, 2, 512], f32, tag="o2", bufs=2, name="ot2")
                for cc in range(C // cw):
                    pso = psA.tile([P, 512], f32, tag="bank", name="pso")
                    for m in range(NHC):
                        nc.tensor.matmul(
                            pso[:, :cw],
                            lhsT=attnT_sb[:, m, ftA * P:(ftA + 1) * P],
                            rhs=wo_sb[:, m, cc * cw:(cc + 1) * cw],
                            start=(m == 0), stop=(m == NHC - 1))
                    nc.vector.tensor_copy(ot[:, cc, :cw], pso[:, :cw])
                nc.sync.dma_start(
                    out_d.ap()[ftA * P:(ftA + 1) * P, :], ot[:])

    nc.compile()
    return nc


_CACHE = {}


def _get_nc():
    if "nc" not in _CACHE:
        _CACHE["nc"] = build_attention_nc(C=C, F=F, T=T, NHEADS=HEADS // HG,
                                          H=DEPTH, use_fp8=USE_FP8_SCORES)
    return _CACHE["nc"]


def kernel(query_input, source_input, bias, wq, wk, wv, wo, **run_kwargs):
    import ml_dtypes
    from concourse.bass_utils import run_bass_kernel_spmd

    bf = ml_dtypes.bfloat16
    q = np.asarray(query_input, dtype=np.float32)
    s = np.asarray(source_input, dtype=np.float32)
    b = np.asarray(bias, dtype=np.float32)
    scale = float(DEPTH) ** -0.5
    wq2 = np.asarray(wq, dtype=np.float32).reshape(C, HEADS * DEPTH) * scale
    wk2 = np.asarray(wk, dtype=np.float32).reshape(C, HEADS * DEPTH)
    wv2 = np.asarray(wv, dtype=np.float32).reshape(C, HEADS * DEPTH)
    wo2 = np.asarray(wo, dtype=np.float32).reshape(HEADS * DEPTH, C)

    qT = [np.ascontiguousarray(q[i].T).astype(bf) for i in range(B)]
    sT = [np.ascontiguousarray(s[i].T).astype(bf) for i in range(B)]
    ebT = np.exp(np.ascontiguousarray(b[0, 0].T)).astype(bf)

    nhl = (HEADS // HG) * DEPTH  # NH columns per core (256)
    # DoubleRow column permutation: psum partition pi of matmul group m must
    # hold nh = (pi//32)*64 + 32*m + pi%32 (head pi//32, k-tile m, row pi%32)
    pi = np.arange(128)
    if USE_FP8_SCORES:
        dr_perm = np.concatenate([(pi // 32) * 64 + 32 * m + pi % 32 for m in (0, 1)])
    else:
        dr_perm = np.arange(2 * 128)
    in_maps = []
    for c in range(N_CORES):
        bi, hg = c // HG, c % HG
        sl = slice(hg * nhl, (hg + 1) * nhl)
        in_maps.append({
            "qT": qT[bi],
            "sT": sT[bi],
            "ebT": ebT,
            "ident": np.eye(128, dtype=np.float32).astype(bf),
            "wq": np.ascontiguousarray(wq2[:, sl][:, dr_perm]).astype(bf),
            "wk": np.ascontiguousarray(wk2[:, sl][:, dr_perm]).astype(bf),
            "wv": np.ascontiguousarray(wv2[:, sl]).astype(bf),
            "wo": np.ascontiguousarray(wo2[sl, :]).astype(bf),
        })

    nc = _get_nc()
    res = run_bass_kernel_spmd(nc, in_maps, core_ids=list(range(N_CORES)), **run_kwargs)
    _CACHE["last_results"] = res

    out = np.empty((B, F, C), np.float32)
    for bi in range(B):
        acc = res.results[bi * HG]["out_p"].astype(np.float32)
        for hg in range(1, HG):
            acc = acc + res.results[bi * HG + hg]["out_p"]
        out[bi] = acc
    return out


# revision 48
# speedup vs baseline: 1.0739x; 1.0171x over previous
# Trainium2 Bass kernel for nn_AttentionLayer_69380901699611.
#
# Full-input contract: kernel(**inputs) takes the unsharded numpy inputs and
# returns the full [B, F, HIDDEN] output. Internally the work is sharded over
# 8 NeuronCores as (batch x head-group): core c handles batch c//4 and heads
# [4*(c%4), 4*(c%4)+4). Each core computes a partial output projection over
# its 4 heads; the host sums the 4 partials per batch.
#
# Device kernel layout (per core):
#   qT, kT      [NH=256 part-chunks, F/T] bf16 (head-dim on partitions)
#   v           [T part, NH] bf16 with an appended ones column per head (the
#               softmax denominator falls out of the attn matmul for free)
#   scores^T    [T part, F free] fp32 psum = kT_chunk.T @ qT_chunk (K=64 pairs
#               on row groups 0-63 / 64-127)
#   softmax     exp on ACT (psum -> bf16), then multiply by exp(bias)^T
#               (precomputed on host, bf16) on DVE in 2x bf16 mode.
#               No max-subtraction needed: |logits| <~ 12.
#   attn        accumulated as [F-tile part, (h, H+1) free] fp32 psum: the
#               attn matmuls use pt (scores^T) as the STATIONARY operand and
#               v as the moving operand, so each matmul is N=65 wide with all
#               128 output partitions used -- half the PE rows of the
#               [nh part, F free] orientation.
#   normalize   per-partition: reciprocal of the denominator column then one
#               broadcast tensor_tensor per F-tile (DVE), bf16 out.
#   transpose   [f, nh] -> [nh, f] via the XBAR DMA transpose (16x128 tiles,
#               runs on the DMA engines, not PE/DVE/ACT).
#   out         attnT.T @ wo per F-tile; psum DMA'd straight to DRAM.

import numpy as np

B, F, T, C = 2, 2048, 2048, 1024
HEADS, DEPTH = 16, 64
N_CORES = 8
HG = 4  # head-groups; heads per group = HEADS // HG = 4
# fp8e4m3 DoubleRow score matmuls: halves score-matmul PE time but raises
# rel err from ~4e-3 to ~1.7e-2 (gate is 2e-2). Off = safe margin.
USE_FP8_SCORES = False


def build_attention_nc(C=1024, F=2048, T=2048, NHEADS=4, H=64, fc_w=512,
                       use_fp8=False, debug_taps=False):
    import concourse.tile as tile
    import concourse.mybir as mybir
    from concourse import bacc

    P = 128
    NH = NHEADS * H          # local heads * depth (256)
    KC = C // P              # contraction subtiles for projections (8)
    NFC = F // fc_w          # F chunks (4)
    NTT = T // P             # T tiles (16)
    NHC = NH // P            # NH chunks of 128 partitions (2)
    FPC = fc_w // P          # F tiles per F chunk (4)
    assert NHC * 2 == NHEADS and H == 64, "layout assumes 2 heads per NH chunk"
    f32 = mybir.dt.float32
    bf16 = mybir.dt.bfloat16
    fp8 = mybir.dt.float8e4
    DR = mybir.MatmulPerfMode.DoubleRow
    Exp = mybir.ActivationFunctionType.Exp
    Mult = mybir.AluOpType.mult

    nc = bacc.Bacc("TRN2", target_bir_lowering=False, debug=False, name="attn69")

    qT_d = nc.dram_tensor("qT", [C, F], bf16, kind="ExternalInput")
    sT_d = nc.dram_tensor("sT", [C, T], bf16, kind="ExternalInput")
    eb_d = nc.dram_tensor("ebT", [T, F], bf16, kind="ExternalInput")
    id_d = nc.dram_tensor("ident", [P, P], bf16, kind="ExternalInput")
    wq_d = nc.dram_tensor("wq", [C, NH], bf16, kind="ExternalInput")
    wk_d = nc.dram_tensor("wk", [C, NH], bf16, kind="ExternalInput")
    wv_d = nc.dram_tensor("wv", [C, NH], bf16, kind="ExternalInput")
    wo_d = nc.dram_tensor("wo", [NH, C], bf16, kind="ExternalInput")
    out_d = nc.dram_tensor("out_p", [F, C], f32, kind="ExternalOutput")

    with tile.TileContext(nc) as tc:
        with (
            tc.tile_pool(name="constp", bufs=1) as constp,
            tc.tile_pool(name="persist", bufs=1) as persist,
            tc.tile_pool(name="actp", bufs=4) as actp,
            tc.tile_pool(name="biasp", bufs=6) as biasp,
            tc.tile_pool(name="ptp", bufs=3) as ptp,
            tc.tile_pool(name="smallp", bufs=4) as smallp,
            tc.tile_pool(name="normp", bufs=6) as normp,
            tc.tile_pool(name="psA", bufs=4, space="PSUM") as psA,
            tc.tile_pool(name="psS", bufs=2, space="PSUM") as psS,
        ):
            # ---------------- weights (tiles only; DMAs ordered below) -------
            wq_sb = constp.tile([P, KC, NH], bf16, name="wq_sb")
            wk_sb = constp.tile([P, KC, NH], bf16, name="wk_sb")
            wv_sb = constp.tile([P, KC, NH], bf16, name="wv_sb")
            wo_sb = constp.tile([P, NHC, C], bf16, name="wo_sb")

            # ---------------- persistent activations ----------------
            # fp8 mode: q/k live in fp8e4m3 with the DoubleRow layout --
            # partition 32*h + p holds head h's contraction rows p and 32+p
            # (kt the second dim); wq/wk columns are pre-permuted on the host
            # so the projection psum lands in exactly this partition order.
            # bf16 mode: q/k live as [head-dim part, pair, F] like v.
            qk_dt = fp8 if use_fp8 else bf16
            qT8 = persist.tile([P, 2, F], qk_dt, name="qT8")
            kT8 = persist.tile([P, 2, T], qk_dt, name="kT8")
            v_sb = persist.tile([P, NTT, NHEADS, H + 1], bf16, name="v_sb")
            attnT_sb = persist.tile([P, NHC, F], bf16, name="attnT_sb")
            # ones column for the softmax denominator (cols 0..H-1 overwritten
            # by the v projection; only col H needs initializing)
            ones1 = nc.const_aps.aps[(f32, 1.0)]
            nc.scalar.copy(
                v_sb[:, :, :, H:H + 1],
                ones1[:, None, None, :].to_broadcast((P, NTT, NHEADS, 1)))

            # ---------------- q projection (emitted per F chunk) ----------------
            # depth**-0.5 is folded into wq on the host, so this is a plain
            # psum->sbuf copy (DVE, keeping ACT free for the exps).
            qT_r = qT_d.ap().rearrange("(ko p) f -> p ko f", p=P)
            sT_r = sT_d.ap().rearrange("(ko p) t -> p ko t", p=P)
            t_per_chunk = fc_w // P

            def load_q(fc):
                qa = actp.tile([P, KC, fc_w], bf16, tag="act", name="qa")
                nc.sync.dma_start(qa[:], qT_r[:, :, fc * fc_w:(fc + 1) * fc_w])
                return qa

            def load_s(sc):
                sa = actp.tile([P, KC, fc_w], bf16, tag="act", name="sa")
                nc.sync.dma_start(sa[:], sT_r[:, :, sc * fc_w:(sc + 1) * fc_w])
                return sa

            def q_proj_m(fc, m, qa):
                psq = psA.tile([P, 512], f32, tag="bank", name="psq")
                for k in range(KC):
                    nc.tensor.matmul(
                        psq[:, :fc_w],
                        lhsT=wq_sb[:, k, m * P:(m + 1) * P],
                        rhs=qa[:, k, :],
                        start=(k == 0), stop=(k == KC - 1))
                nc.vector.tensor_copy(
                    qT8[:, m, fc * fc_w:(fc + 1) * fc_w], psq[:, :fc_w])

            def q_proj(fc, qa=None):
                if qa is None:
                    qa = load_q(fc)
                for m in range(NHC):
                    q_proj_m(fc, m, qa)

            # ---------------- k and v projections ----------------
            def k_proj(sc, sa):
                for m in range(NHC):
                    psk = psA.tile([P, 512], f32, tag="bank", name="psk")
                    for k in range(KC):
                        nc.tensor.matmul(
                            psk[:, :fc_w],
                            lhsT=wk_sb[:, k, m * P:(m + 1) * P],
                            rhs=sa[:, k, :],
                            start=(k == 0), stop=(k == KC - 1))
                    nc.vector.tensor_copy(kT8[:, m, sc * fc_w:(sc + 1) * fc_w], psk[:, :fc_w])

            def v_proj(sc, sa):
                for tl in range(t_per_chunk):
                    tt = sc * t_per_chunk + tl
                    psv = psA.tile([P, 512], f32, tag="bank", name="psv")
                    for k in range(KC):
                        nc.tensor.matmul(
                            psv[:, :NH],
                            lhsT=sa[:, k, tl * P:(tl + 1) * P],
                            rhs=wv_sb[:, k, :],
                            start=(k == 0), stop=(k == KC - 1))
                    nc.vector.tensor_copy(
                        v_sb[:, tt, :, 0:H],
                        psv[:, :NH].rearrange("p (h x) -> p h x", h=NHEADS))

            # ------------- attention main loop (software-pipelined) -------------
            # Chunk fc's softmax stream (ST matmuls -> exp -> *exp(bias))
            # produces NTT pt tiles; chunk fc-1's attention accumulation,
            # normalize, and output projection are interleaved with it. Chunk
            # 0's stream overlaps the k/v projection prefix, so ACT/DVE are
            # busy during the PE-dense projection phase and across chunk
            # boundaries.
            pt_store = {}
            bias_store = {}
            eb_r = eb_d.ap().rearrange("(tg p) f -> p tg f", p=P)

            def load_b4(fc, q):
                # one bias DMA per quarter chunk (four t-tiles)
                b4 = biasp.tile([P, 4, fc_w], bf16, tag="bias", name="b4")
                nc.sync.dma_start(
                    b4[:], eb_r[:, 4 * q:4 * q + 4, fc * fc_w:(fc + 1) * fc_w])
                bias_store[(fc, q)] = b4

            def produce(fc, tt):
                fsl = slice(fc * fc_w, (fc + 1) * fc_w)
                if (fc, tt // 4) not in bias_store:
                    load_b4(fc, tt // 4)
                bias_t = bias_store[(fc, tt // 4)][:, tt % 4, :]
                if tt % 4 == 3:
                    del bias_store[(fc, tt // 4)]
                pt4 = ptp.tile([P, NHEADS, fc_w], bf16, tag="pt", bufs=18, name="pt4")
                for pair in range(NHC):
                    st2 = psS.tile([P, 2, 512], f32, tag="st", name="st2")
                    for j in range(2):
                        h = 2 * pair + j
                        if use_fp8:
                            nc.tensor.matmul(
                                st2[:, j, :fc_w],
                                lhsT=kT8[32 * h:32 * h + 32, :, tt * P:(tt + 1) * P],
                                rhs=qT8[32 * h:32 * h + 32, :, fsl],
                                start=True, stop=True, perf_mode=DR,
                                tile_position=(32 * h, 0))
                        else:
                            off = j * H
                            nc.tensor.matmul(
                                st2[:, j, :fc_w],
                                lhsT=kT8[off:off + H, pair, tt * P:(tt + 1) * P],
                                rhs=qT8[off:off + H, pair, fsl],
                                start=True, stop=True)
                    # exp(S^T), psum fp32 -> bf16
                    nc.scalar.activation(
                        pt4[:, 2 * pair:2 * pair + 2, :], st2[:, :, :fc_w], Exp)
                # multiply by exp(bias)^T tile, bf16 2x mode; bias is
                # broadcast over the head dim (stride-0): one instr per tile
                nc.vector.tensor_mul(
                    pt4[:], pt4[:],
                    bias_t[:, None, :].to_broadcast((P, NHEADS, fc_w)))
                pt_store[(fc, tt)] = pt4

            def consume(fc, tt, at_tiles):
                # at_tiles[ft] accumulates [128 f, (h, H+1)] for F-tile ft;
                # pt is the stationary operand so each matmul is only N=65.
                # The bank is pre-zeroed by alloc_at and every matmul runs
                # with start=False: a start=True in a bank discards any other
                # open accumulation group's partials there (measured on HW),
                # so per-head start flags cannot share a bank.
                pt4 = pt_store.pop((fc, tt))
                for ft in range(FPC):
                    at = at_tiles[ft]
                    for h in range(NHEADS):
                        nc.tensor.matmul(
                            at[:, h, :],
                            lhsT=pt4[:, h, ft * P:(ft + 1) * P],
                            rhs=v_sb[:, tt, h, :],
                            start=False, stop=(tt == NTT - 1),
                            skip_group_check=True)

            cw = min(512, C)

            def norm_ft(fc, ft, at):
                # normalize: attn[f, h, :] = acc[f, h, 0:H] / acc[f, h, H]
                rec = smallp.tile([P, NHEADS], f32, tag="rec", name="rec")
                nc.vector.reciprocal(rec[:], at[:, :, H])
                an = normp.tile([P, NHEADS, H], bf16, tag="an", name="an")
                nc.vector.tensor_tensor(
                    an[:], at[:, :, 0:H],
                    rec[:, :, None].to_broadcast((P, NHEADS, H)), Mult)
                return an

            def tail_ft(fc, ft, an):
                # XBAR-DMA-transpose the [128 f, 128 nh] blocks into attnT_sb,
                # then this F-tile's output projection
                ftA = fc * FPC + ft
                for m in range(NHC):
                    nc.sync.dma_start_transpose(
                        attnT_sb[:, m, ftA * P:(ftA + 1) * P],
                        an[:, 2 * m:2 * m + 2, :])
                for cc in range(C // cw):
                    pso = psA.tile([P, 512], f32, tag="bank", name="pso")
                    for m in range(NHC):
                        nc.tensor.matmul(
                            pso[:, :cw],
                            lhsT=attnT_sb[:, m, ftA * P:(ftA + 1) * P],
                            rhs=wo_sb[:, m, cc * cw:(cc + 1) * cw],
                            start=(m == 0), stop=(m == NHC - 1))
                    ot = normp.tile([P, 512], f32, tag="o", name="ot")
                    nc.vector.tensor_copy(ot[:, :cw], pso[:, :cw])
                    nc.sync.dma_start(
                        out_d.ap()[ftA * P:(ftA + 1) * P, cc * cw:(cc + 1) * cw],
                        ot[:, :cw])

            def make_piece(fc, ft, cc, an):
                # one drip-feedable slice of finish: the F-tile's transposes
                # (first slice only) plus one outproj psum + copy + store
                def piece():
                    ftA = fc * FPC + ft
                    if cc == 0:
                        for m in range(NHC):
                            nc.sync.dma_start_transpose(
                                attnT_sb[:, m, ftA * P:(ftA + 1) * P],
                                an[:, 2 * m:2 * m + 2, :])
                    pso = psA.tile([P, 512], f32, tag="bank", name="pso")
                    for m in range(NHC):
                        nc.tensor.matmul(
                            pso[:, :cw],
                            lhsT=attnT_sb[:, m, ftA * P:(ftA + 1) * P],
                            rhs=wo_sb[:, m, cc * cw:(cc + 1) * cw],
                            start=(m == 0), stop=(m == NHC - 1))
                    ot = normp.tile([P, 512], f32, tag="o", bufs=6, name="ot")
                    nc.vector.tensor_copy(ot[:, :cw], pso[:, :cw])
                    nc.sync.dma_start(
                        out_d.ap()[ftA * P:(ftA + 1) * P, cc * cw:(cc + 1) * cw],
                        ot[:, :cw])
                return piece

            # PE p-state warm-up: ~45 dummy matmuls on scratch data keep the
            # PE continuously busy from t~0.7us so the first real projection
            # matmuls run at the full 2.4 GHz clock instead of 0.65/1.2 GHz.
            scr = smallp.tile([P, 512], bf16, tag="scr", name="scr")
            nc.gpsimd.memset(scr[:], 0.0)
            pswu = psA.tile([P, 512], f32, tag="bank", name="pswu")
            for _ in range(45):
                nc.tensor.matmul(
                    pswu[:, :128], lhsT=scr[:, 0:128], rhs=scr[:, 128:256],
                    start=True, stop=True, skip_group_check=True)

            # chunk 0 production rides along with the k/v projections.
            # DMA issue order is tuned for the serial DMA pipe: k-proj inputs
            # first (sa half, wk), then q (qa, wq), then the first bias
            # quarter and wv; wo (needed ~60us in) goes last.
            sa0 = actp.tile([P, KC, fc_w], bf16, tag="act", name="sa0")
            nc.sync.dma_start(sa0[:, 0:KC // 2, :], sT_r[:, 0:KC // 2, 0:fc_w])
            nc.sync.dma_start(
                wk_sb[:], wk_d.ap().rearrange("(ko p) m -> p ko m", p=P))
            nc.sync.dma_start(sa0[:, KC // 2:, :], sT_r[:, KC // 2:, 0:fc_w])
            qa0 = actp.tile([P, KC, fc_w], bf16, tag="act", name="qa0")
            nc.sync.dma_start(qa0[:, 0:KC // 2, :], qT_r[:, 0:KC // 2, 0:fc_w])
            nc.sync.dma_start(
                wq_sb[:], wq_d.ap().rearrange("(ko p) m -> p ko m", p=P))
            nc.sync.dma_start(qa0[:, KC // 2:, :], qT_r[:, KC // 2:, 0:fc_w])
            load_b4(0, 0)
            nc.sync.dma_start(
                wv_sb[:], wv_d.ap().rearrange("(ko p) m -> p ko m", p=P))
            nc.sync.dma_start(
                wo_sb[:], wo_d.ap().rearrange("(ko p) m -> p ko m", p=P))
            id_sb = constp.tile([P, P], bf16, name="id_sb")
            nc.sync.dma_start(id_sb[:], id_d.ap())

            # emission order puts each source chunk's k-projection and the
            # score/exp stream before its v-projection (v is only needed by
            # the much-later consume stage)
            sa_next = None
            for sc in range(T // fc_w):
                sa = sa0 if sc == 0 else sa_next
                k_proj(sc, sa)
                if sc == 0:
                    q_proj(0, qa0)
                if sc + 1 < T // fc_w:
                    load_b4(0, sc + 1)
                    sa_next = load_s(sc + 1)
                for tl in range(t_per_chunk):
                    produce(0, sc * t_per_chunk + tl)
                v_proj(sc, sa)
            # q(1) is needed immediately by the main loop; q(2)/q(3) are
            # dripped into the first main-loop slots (one m-group per slot)
            q_proj(1)
            deferred_q = []
            for fc in (2, 3):
                qa = load_q(fc)
                for m in range(NHC):
                    deferred_q.append((fc, m, qa))

            def alloc_at():
                tiles = []
                for ft in range(FPC):
                    at = psA.tile([P, NHEADS, H + 1], f32, tag="bank", name=f"at{ft}")
                    nc.vector.memset(at[:], 0.0)
                    tiles.append(at)
                return tiles

            # Greedy catch-up pipeline: the produce stream runs continuously
            # (chunks 1..NFC-1) while consumes trail, draining at up to two
            # units per produce slot. A unit is either a consume tile or a
            # finish "piece" (transpose + one outproj psum): finish work is
            # drip-fed across slots so it never bursts the PE at a chunk
            # boundary, and piece psums recycle the accumulator banks BEFORE
            # the next chunk's memsets claim them (pool request order).
            at_cur = alloc_at()  # chunk 0 accumulators
            c = 0          # global consume pointer (tile index)
            produced = NTT  # chunk 0 fully produced in the prefix
            pending = []   # queued finish pieces
            need_alloc = False

            def emit_slot_work(budget):
                nonlocal c, at_cur, need_alloc
                while budget > 0:
                    if pending:
                        pending.pop(0)()
                        budget -= 1
                        continue
                    if need_alloc:
                        at_cur = alloc_at()
                        need_alloc = False
                    if at_cur is None or c >= produced - 1:
                        break
                    cfc, ctt = divmod(c, NTT)
                    consume(cfc, ctt, at_cur)
                    c += 1
                    budget -= 1
                    if ctt == NTT - 1:
                        ans = [norm_ft(cfc, ft, at_cur[ft]) for ft in range(FPC)]
                        for ft in range(FPC):
                            for cc in range(C // cw):
                                pending.append(make_piece(cfc, ft, cc, ans[ft]))
                        at_cur = None
                        need_alloc = cfc + 1 < NFC

            for fc in range(1, NFC):
                for tt in range(NTT):
                    produce(fc, tt)
                    produced += 1
                    if deferred_q:
                        q_proj_m(*deferred_q.pop(0))
                        emit_slot_work(1)
                    else:
                        emit_slot_work(2)
            # Tail: flush pending pieces and stragglers, then drain the last
            # chunk F-tile-major so each F-tile's normalize/transpose/outproj
            # overlaps the next F-tile's accumulation matmuls.
            while pending:
                pending.pop(0)()
            if need_alloc:
                at_cur = alloc_at()
                need_alloc = False
            while c < (NFC - 1) * NTT:
                cfc, ctt = divmod(c, NTT)
                consume(cfc, ctt, at_cur)
                c += 1
                if ctt == NTT - 1:
                    ans = [norm_ft(cfc, ft, at_cur[ft]) for ft in range(FPC)]
                    for ft in range(FPC):
                        tail_ft(cfc, ft, ans[ft])
                    at_cur = alloc_at()
            # Stage-parallel last-chunk epilogue: all consume matmuls, then
            # all norms (DVE), then all PE-transposes + ACT copies (identity
            # matmul keeps the chain off the serial HWDGE/DMA pipe), then the
            # outprojs with psum->sbuf copies alternating DVE/ACT and per-cc
            # output DMAs so the serial DMA device starts draining early.
            last = NFC - 1
            rem = [divmod(g, NTT)[1] for g in range(c, NFC * NTT)]
            pts = {tt: pt_store.pop((last, tt)) for tt in rem}
            for ft in range(FPC):
                at = at_cur[ft]
                for tt in rem:
                    for h in range(NHEADS):
                        nc.tensor.matmul(
                            at[:, h, :],
                            lhsT=pts[tt][:, h, ft * P:(ft + 1) * P],
                            rhs=v_sb[:, tt, h, :],
                            start=False, stop=(tt == rem[-1]),
                            skip_group_check=True)
            ans = [norm_ft(last, ft, at_cur[ft]) for ft in range(FPC)]
            for ft in range(FPC):
                ftA = last * FPC + ft
                pst = psA.tile([P, 512], f32, tag="bank", name="pst")
                pst_b = pst[:].bitcast(bf16)
                for m in range(NHC):
                    nc.tensor.matmul(
                        pst_b[:, m * P:(m + 1) * P],
                        lhsT=ans[ft][:, 2 * m:2 * m + 2, :],
                        rhs=id_sb[:],
                        start=True, stop=True, is_transpose=True,
                        skip_group_check=True)
                    nc.scalar.copy(
                        attnT_sb[:, m, ftA * P:(ftA + 1) * P],
                        pst_b[:, m * P:(m + 1) * P])
            for ft in range(FPC):
                ftA = last * FPC + ft
                for cc in range(C // cw):
                    pso = psA.tile([P, 512], f32, tag="bank", name="pso")
                    for m in range(NHC):
                        nc.tensor.matmul(
                            pso[:, :cw],
                            lhsT=attnT_sb[:, m, ftA * P:(ftA + 1) * P],
                            rhs=wo_sb[:, m, cc * cw:(cc + 1) * cw],
                            start=(m == 0), stop=(m == NHC - 1))
                    ot = normp.tile([P, 512], f32, tag="o", bufs=6, name="ot")
                    if (2 * ft + cc) % 2 == 0:
                        nc.vector.tensor_copy(ot[:, :cw], pso[:, :cw])
                    else:
                        nc.scalar.copy(ot[:, :cw], pso[:, :cw])
                    nc.sync.dma_start(
                        out_d.ap()[ftA * P:(ftA + 1) * P, cc * cw:(cc + 1) * cw],
                        ot[:, :cw])

    nc.compile()
    return nc


_CACHE = {}


def _get_nc():
    if "nc" not in _CACHE:
        _CACHE["nc"] = build_attention_nc(C=C, F=F, T=T, NHEADS=HEADS // HG,
                                          H=DEPTH, use_fp8=USE_FP8_SCORES)
    return _CACHE["nc"]


def kernel(query_input, source_input, bias, wq, wk, wv, wo, **run_kwargs):
    import ml_dtypes
    from concourse.bass_utils import run_bass_kernel_spmd

    bf = ml_dtypes.bfloat16
    q = np.asarray(query_input, dtype=np.float32)
    s = np.asarray(source_input, dtype=np.float32)
    b = np.asarray(bias, dtype=np.float32)
    scale = float(DEPTH) ** -0.5
    wq2 = np.asarray(wq, dtype=np.float32).reshape(C, HEADS * DEPTH) * scale
    wk2 = np.asarray(wk, dtype=np.float32).reshape(C, HEADS * DEPTH)
    wv2 = np.asarray(wv, dtype=np.float32).reshape(C, HEADS * DEPTH)
    wo2 = np.asarray(wo, dtype=np.float32).reshape(HEADS * DEPTH, C)

    qT = [np.ascontiguousarray(q[i].T).astype(bf) for i in range(B)]
    sT = [np.ascontiguousarray(s[i].T).astype(bf) for i in range(B)]
    ebT = np.exp(np.ascontiguousarray(b[0, 0].T)).astype(bf)

    nhl = (HEADS // HG) * DEPTH  # NH columns per core (256)
    # DoubleRow column permutation: psum partition pi of matmul group m must
    # hold nh = (pi//32)*64 + 32*m + pi%32 (head pi//32, k-tile m, row pi%32)
    pi = np.arange(128)
    if USE_FP8_SCORES:
        dr_perm = np.concatenate([(pi // 32) * 64 + 32 * m + pi % 32 for m in (0, 1)])
    else:
        dr_perm = np.arange(2 * 128)
    in_maps = []
    for c in range(N_CORES):
        bi, hg = c // HG, c % HG
        sl = slice(hg * nhl, (hg + 1) * nhl)
        in_maps.append({
            "qT": qT[bi],
            "sT": sT[bi],
            "ebT": ebT,
            "ident": np.eye(128, dtype=np.float32).astype(bf),
            "wq": np.ascontiguousarray(wq2[:, sl][:, dr_perm]).astype(bf),
            "wk": np.ascontiguousarray(wk2[:, sl][:, dr_perm]).astype(bf),
            "wv": np.ascontiguousarray(wv2[:, sl]).astype(bf),
            "wo": np.ascontiguousarray(wo2[sl, :]).astype(bf),
        })

    nc = _get_nc()
    res = run_bass_kernel_spmd(nc, in_maps, core_ids=list(range(N_CORES)), **run_kwargs)
    _CACHE["last_results"] = res

    out = np.empty((B, F, C), np.float32)
    for bi in range(B):
        acc = res.results[bi * HG]["out_p"].astype(np.float32)
        for hg in range(1, HG):
            acc = acc + res.results[bi * HG + hg]["out_p"]
        out[bi] = acc
    return out
